# revision 3
# baseline (speedup 1.0000x reference)
"""TRN2 Bass kernel for nn_Attention_369367187796 — Gram-route scores.

Reference (B=4, DX=1024, N=4096, DQ=DK=DV=1024, fp32):
    Q = Wq @ x[b]; K = Wk @ x[b]; V = Wv @ x[b]
    scores = Q @ K.T   (contract n)
    p = softmax(scores / sqrt(DQ), axis=q)   <- softmax over q
    out[q,n] = sum_k p[q,k] V[k,n]

Key algebra: Q and K are used ONLY in scores, and
    scores = Wq (x x^T) Wk^T
so per batch the Q/K/scores path costs 4.3+1.1+1.1 GMAC via the Gram matrix
G = x x^T instead of 12.9 GMAC for Qproj+Kproj+scores. With softmax over q,
any score term constant across q drops out, so mean-removed weights
(Wq' = Wq-0.5, Wk' = Wk-0.5) need only the rank-1 q-varying correction
    t[q] = 0.5 * sum_d Wq'[q,d] g[d],   g = rowsum(G)
restored (two f32r limbs; K-side and const terms cancel in softmax).

Precision (numpy-simulated; sim matches HW for the old direct scheme to
1e-5): single-limb f32r everywhere (G, A=Wq'G, Wk', V, p) gives end-to-end
rel err ~8e-4 vs fp64 (HW-verified) — 25x under the 2e-2 gate.

Sharding: 8 cores = 4 batches x 2 k-halves (DKH=512). Each core computes the
full G/A (duplicated within the pair), its k-half of scores/softmax/V, and
partial out[q,n] summed on the host — no cross-core communication.

Per-core phases:
  A: stream x (256-col chunks), PE-transpose raw f32 -> xT (f32r on evict);
     the first G generation (rows 0-3 x cols 0:512) accumulates in a
     dedicated 4-bank PSUM pool as xT tiles land, keeping PE dense
  B: remaining G generations (32-matmul PSUM chains), mirror lower-left via
     PE-transpose (G symmetric), g = rowsum(G)
  C: A^T[d',q] = G Wq'^T; scoresT[k,q] = Wk'^T_half A^T + ones x (th+tl);
     softmax over q straight from the score PSUMs
  E: stream x again: V k-half projection fused with out = p^T V, DMA out
"""

import math

import numpy as np

B_FULL, DX_FULL, N_FULL = 4, 1024, 4096
DQ_FULL = DK_FULL = 1024
N_CORES = 8


def _build_core_kernel(DX, N, DQ, DKH, bench=False, bench_reps=0):
    import concourse.bass as bass
    import concourse.mybir as mybir
    import concourse.tile as tile
    from concourse import bacc

    f32 = mybir.dt.float32
    f32r = mybir.dt.float32r

    P = 128
    DT = DX // P            # 8 d-tiles (also d' tiles)
    NT = N // P             # 32 n-tiles
    CHA = 256               # phase A chunk cols
    NCA = N // CHA          # 16 chunks
    ECH = int(__import__("os").environ.get("GRAM_ECH", "256"))
    NCE = N // ECH          # 16 chunks
    KT = DKH // P           # 4 k-tiles
    QT128 = DQ // P         # 8 q-tiles
    QC = DQ // 512          # 2 q-chunks
    scale = 1.0 / math.sqrt(DQ)

    assert DX % P == 0 and N % ECH == 0 and DQ % 512 == 0 and DKH % P == 0

    nc = bacc.Bacc(None, target_bir_lowering=False, debug=False)

    kind_big = "Internal" if bench else "ExternalInput"
    # In bench mode out is Internal (only [1,1] seed/sink cross the tunnel);
    # the post-loop readback of out[0,0] into sink keeps every out write
    # live — without it the compiler dead-code-eliminates most of phase E
    # (observed as a physically impossible 174us/iter).
    kind_out = "Internal" if bench else "ExternalOutput"
    xb = nc.dram_tensor("xb", [DX, N], f32, kind=kind_big)
    wqt = nc.dram_tensor("wqt", [DX, DQ], f32, kind=kind_big)
    wkt = nc.dram_tensor("wkt", [DX, DKH], f32, kind=kind_big)
    wvt = nc.dram_tensor("wvt", [DX, DKH], f32, kind=kind_big)
    # identity for PE transposes: tiny, stays ExternalInput in bench mode too
    ident = nc.dram_tensor("ident", [P, P], f32, kind="ExternalInput")
    seed = nc.dram_tensor("seed", [1, 1], f32, kind="ExternalInput")
    out = nc.dram_tensor("out", [DQ, N], f32, kind=kind_out)
    sink = (nc.dram_tensor("sink", [1, 1], f32, kind="ExternalOutput")
            if bench else None)

    xv = xb.ap().rearrange("(dt p) n -> p dt n", p=P)
    wqv = wqt.ap().rearrange("(dt p) q -> p dt q", p=P)
    wkv = wkt.ap().rearrange("(dt p) k -> p dt k", p=P)
    wvv = wvt.ap().rearrange("(dt p) k -> p dt k", p=P)

    with tile.TileContext(nc) as tc:
        with (
            tc.tile_pool(name="ps", bufs=4, space="PSUM") as ps,
            tc.tile_pool(name="psg", bufs=4, space="PSUM") as psg,
        ):
            p0_cm = tc.tile_pool(name="pres0", bufs=1)
            p0 = p0_cm.__enter__()
            ident_r = p0.tile([P, P], f32r, tag="idr", name="ident_r")
            ident_f = p0.tile([P, P], f32, tag="idf", name="ident_f")
            ones_row = p0.tile([1, P], f32r, tag="ones", name="ones_row")
            g_hi = p0.tile([P, DT, 1], f32r, tag="ghi", name="g_hi")
            g_lo = p0.tile([P, DT, 1], f32r, tag="glo", name="g_lo")

            rep_cm = tc.For_i(0, bench_reps, 1) if bench_reps else None
            if rep_cm is not None:
                rep_cm.__enter__()

            # engine rotation for PSUM->SBUF evictions (GPSIMD can't
            # read PSUM, so alternate DVE and Act)
            def evict(i, dst, src):
                if i % 2 == 0:
                    nc.vector.tensor_copy(dst, src)
                else:
                    nc.scalar.copy(dst, src)

            # prologue: identity + ones (f32r via rounding compute)
            with tc.tile_pool(name="ppro", bufs=1) as ppro:
                istage = ppro.tile([P, P], f32, tag="ist", name="istage")
                nc.sync.dma_start(istage[:], ident.ap())
                nc.sync.dma_start(ident_f[:], ident.ap())
                nc.vector.tensor_copy(ident_r[:], istage[:])
                nc.gpsimd.memset(istage[:, 0:P], 1.0)
                nc.vector.tensor_copy(ones_row[:], istage[0:1, 0:P])

            # ------- Phase A: xT = round(x)^T; G gen0 chases the chunks ----
            pg_cm = tc.tile_pool(name="pg", bufs=1)
            pg = pg_cm.__enter__()
            g_sb = pg.tile([P, DT, DX], f32r, tag="g", name="g_sb")

            # first Wq' quarter prefetched at body start so A-matmuls can
            # begin the moment G completes (the rest of Wq' loads into pc,
            # whose SBUF region only frees when xT dies)
            pwq0_cm = tc.tile_pool(name="pwq0", bufs=1)
            pwq0 = pwq0_cm.__enter__()
            wq_q0 = pwq0.tile([P, DT, 256], f32r, tag="wq0", name="wq_q0")

            pxt_cm = tc.tile_pool(name="pxt", bufs=1)
            pxt = pxt_cm.__enter__()
            xt = pxt.tile([P, NT, DX], f32r, tag="xt", name="xt")

            # gen0: G rows 0-3 x cols 0:512, accumulated as chunks land
            gp0 = [psg.tile([P, 512], f32, tag="psg", name=f"gp0_{m}")
                   for m in range(4)]

            ei = 0
            with (
                tc.tile_pool(name="pxa", bufs=3) as pxa,
                tc.tile_pool(name="pwq0st", bufs=1) as pwq0st,
            ):
                for c in range(NCA):
                    ncol = bass.ds(c * CHA, CHA)
                    xc = pxa.tile([P, DT, CHA], f32, tag="xc", name=f"xc{c}")
                    nc.sync.dma_start(xc[:], xv[:, :, ncol])
                    if c == 1:
                        w0tmp = pwq0st.tile([P, DT, 256], f32, tag="w0t",
                                            name="w0t")
                        nc.scalar.dma_start(w0tmp[:], wqv[:, :, 0:256])
                        nc.gpsimd.tensor_copy(wq_q0[:], w0tmp[:])
                    for j in range(CHA // P):
                        nt = c * (CHA // P) + j
                        for dh in range(DT // 4):
                            tp = ps.tile([P, 512], f32, tag="ps",
                                         name=f"tp{nt}_{dh}")
                            for di in range(4):
                                dt = dh * 4 + di
                                nc.tensor.transpose(
                                    tp[:, di * P:(di + 1) * P],
                                    xc[:, dt, j * P:(j + 1) * P],
                                    ident_f[:])
                            evict(ei, xt[:, nt, dh * 512:(dh + 1) * 512],
                                  tp[:])
                            ei += 1
                        for m in range(4):
                            nc.tensor.matmul(
                                gp0[m][:], xt[:, nt, m * P:(m + 1) * P],
                                xt[:, nt, 0:512],
                                start=(nt == 0), stop=(nt == NT - 1))

            # ------- Phase B: remaining G generations, mirror, g ----------
            for m in range(4):
                evict(ei, g_sb[:, m, 0:512], gp0[m][:])
                ei += 1
            for rh, chalf in [(0, 1), (1, 1)]:
                csl = bass.ds(chalf * 512, 512)
                for m in range(4):
                    row = rh * 4 + m
                    gp = ps.tile([P, 512], f32, tag="ps",
                                 name=f"gp{row}_{chalf}")
                    for nt in range(NT):
                        nc.tensor.matmul(
                            gp[:], xt[:, nt, row * P:(row + 1) * P],
                            xt[:, nt, csl],
                            start=(nt == 0), stop=(nt == NT - 1))
                    evict(ei, g_sb[:, row, csl], gp[:])
                    ei += 1
            pxt_cm.__exit__(None, None, None)  # free xT (16MB)

            # mirror lower-left: G[4+j, m*128:] = G[m, 512+j*128:]^T
            for j in range(4):
                mp = ps.tile([P, 512], f32r, tag="ps", name=f"mp{j}")
                for m in range(4):
                    nc.tensor.transpose(
                        mp[:, m * P:(m + 1) * P],
                        g_sb[:, m, 512 + j * P:512 + (j + 1) * P],
                        ident_r[:])
                evict(ei, g_sb[:, 4 + j, 0:512], mp[:])
                ei += 1

            # g = rowsum(G) in two f32r limbs
            with tc.tile_pool(name="pgst", bufs=1) as pgst:
                g_f = pgst.tile([P, DT, 1], f32, tag="gf", name="g_f")
                for dt in range(DT):
                    nc.vector.reduce_sum(g_f[:, dt], g_sb[:, dt],
                                         axis=mybir.AxisListType.X)
                nc.vector.tensor_copy(g_hi[:], g_f[:])
                nc.vector.tensor_sub(g_lo[:], g_f[:], g_hi[:])

            # ------- Phase C: A, t, scores, softmax-from-PSUM -------------
            # ppr: tiles that outlive pc (p_r, wv_r used in E; wv prefetch
            # overlaps C because its buffer can't collide with pc tiles)
            ppr_cm = tc.tile_pool(name="ppr", bufs=1)
            ppr = ppr_cm.__enter__()
            p_r = [ppr.tile([P, DQ], f32r, tag=f"pr{kt}", name=f"p{kt}")
                   for kt in range(KT)]
            wv_r = ppr.tile([P, DT, DKH], f32r, tag="wv", name="wv_r")
            xe0 = ppr.tile([P, DT, ECH], f32, tag="xe0", name="xe0")
            nc.sync.dma_start(xe0[:], xv[:, :, 0:ECH])

            pc_cm = tc.tile_pool(name="pc", bufs=1)
            pc = pc_cm.__enter__()
            wq_r = pc.tile([P, DT, DQ - 256], f32r, tag="wq", name="wq_r")
            wk_r = pc.tile([P, DT, DKH], f32r, tag="wk", name="wk_r")
            a_sb = pc.tile([P, DT, DQ], f32r, tag="a", name="a_sb")
            t_hi = pc.tile([1, DQ], f32r, tag="thi", name="t_hi")
            t_lo = pc.tile([1, DQ], f32r, tag="tlo", name="t_lo")

            with tc.tile_pool(name="pwst", bufs=2) as pwst:
                def wq_ap(dt, q0, q1):
                    # Wq' col range [q0:q1): quarter 0 lives in wq_q0,
                    # the rest in wq_r at offset-256
                    if q1 <= 256:
                        return wq_q0[:, dt, q0:q1]
                    return wq_r[:, dt, q0 - 256:q1 - 256]

                for qq in range(1, DQ // 256):
                    wtmp = pwst.tile([P, DT, 256], f32, tag="wt",
                                     name=f"wt{qq}")
                    (nc.sync, nc.scalar, nc.gpsimd)[qq % 3].dma_start(
                        wtmp[:], wqv[:, :, qq * 256:(qq + 1) * 256])
                    nc.vector.tensor_copy(
                        wq_r[:, :, (qq - 1) * 256:qq * 256], wtmp[:])
                for dt in range(0, DT, 2):
                    d2 = bass.ds(dt, 2)
                    wtmp2 = pwst.tile([P, 2, DKH], f32, tag="wt2",
                                      name=f"wt2{dt}")
                    (nc.scalar, nc.gpsimd)[(dt // 2) % 2].dma_start(
                        wtmp2[:], wkv[:, d2])
                    nc.gpsimd.tensor_copy(wk_r[:, d2], wtmp2[:])
                    wtmp3 = pwst.tile([P, 2, DKH], f32, tag="wt3",
                                      name=f"wt3{dt}")
                    (nc.gpsimd, nc.sync)[(dt // 2) % 2].dma_start(
                        wtmp3[:], wvv[:, d2])
                    nc.gpsimd.tensor_copy(wv_r[:, d2], wtmp3[:])

                # A^T[d', q] = sum_d G[d, d'] Wq'^T[d, q], in 256-wide
                # quarters: quarter 0 uses the prefetched wq_q0 and starts
                # right at G-end, overlapping the wq_r load
                for qq in range(DQ // 256):
                    for dpt in range(DT):
                        dsl = bass.ds(dpt * P, P)
                        ap_ = ps.tile([P, 256], f32, tag="ps",
                                      name=f"ap{dpt}_{qq}")
                        for dt in range(DT):
                            nc.tensor.matmul(
                                ap_[:], g_sb[:, dt, dsl],
                                wq_ap(dt, qq * 256, (qq + 1) * 256),
                                start=(dt == 0), stop=(dt == DT - 1))
                        evict(ei, a_sb[:, dpt, bass.ds(qq * 256, 256)],
                              ap_[:])
                        ei += 1

                # t[q] = 0.5 * (g_hi + g_lo)^T Wq'  (two f32r limbs)
                for qq in range(DQ // 256):
                    qsl = bass.ds(qq * 256, 256)
                    tp2 = ps.tile([P, 256], f32, tag="ps", name=f"tq{qq}")
                    first = True
                    for limb in (g_hi, g_lo):
                        for dt in range(DT):
                            nc.tensor.matmul(
                                tp2[0:1, :], limb[:, dt],
                                wq_ap(dt, qq * 256, (qq + 1) * 256),
                                start=first,
                                stop=(limb is g_lo and dt == DT - 1))
                            first = False
                    nc.scalar.mul(t_hi[:, qsl], tp2[0:1, :], 0.5)
                    nc.vector.scalar_tensor_tensor(
                        t_lo[:, qsl], tp2[0:1, :], 0.5, t_hi[:, qsl],
                        op0=mybir.AluOpType.mult,
                        op1=mybir.AluOpType.subtract)

            with (
                tc.tile_pool(name="psmx", bufs=2) as psmx,
                tc.tile_pool(name="pstat", bufs=4) as pstat,
            ):
                # scoresT[k, q] = Wk'^T A^T + ones x (t_hi + t_lo);
                # softmax over q straight from the two q-chunk PSUMs
                for kt in range(KT):
                    ksl = bass.ds(kt * P, P)
                    sp = []
                    for qc in range(QC):
                        qsl = bass.ds(qc * 512, 512)
                        s = ps.tile([P, 512], f32, tag="ps",
                                    name=f"sp{kt}_{qc}")
                        for dpt in range(DT):
                            nc.tensor.matmul(
                                s[:], wk_r[:, dpt, ksl], a_sb[:, dpt, qsl],
                                start=(dpt == 0), stop=False)
                        nc.tensor.matmul(s[:], ones_row[:], t_hi[:, qsl],
                                         start=False, stop=False)
                        nc.tensor.matmul(s[:], ones_row[:], t_lo[:, qsl],
                                         start=False, stop=True)
                        sp.append(s)
                    m0 = pstat.tile([P, 1], f32, tag="m0")
                    m1 = pstat.tile([P, 1], f32, tag="m1")
                    negm = pstat.tile([P, 1], f32, tag="negm")
                    den0 = pstat.tile([P, 1], f32, tag="den0")
                    den1 = pstat.tile([P, 1], f32, tag="den1")
                    rden = pstat.tile([P, 1], f32, tag="rden")
                    nc.vector.reduce_max(m0[:], sp[0][:],
                                         axis=mybir.AxisListType.X)
                    nc.vector.reduce_max(m1[:], sp[1][:],
                                         axis=mybir.AxisListType.X)
                    nc.vector.tensor_max(m0[:], m0[:], m1[:])
                    nc.vector.tensor_scalar_mul(negm[:], m0[:], -scale)
                    e0 = psmx.tile([P, 512], f32, tag="e0")
                    e1 = psmx.tile([P, 512], f32, tag="e1")
                    nc.scalar.activation(
                        e0[:], sp[0][:], mybir.ActivationFunctionType.Exp,
                        bias=negm[:], scale=scale, accum_out=den0[:])
                    nc.scalar.activation(
                        e1[:], sp[1][:], mybir.ActivationFunctionType.Exp,
                        bias=negm[:], scale=scale, accum_out=den1[:])
                    nc.vector.tensor_add(den0[:], den0[:], den1[:])
                    nc.vector.reciprocal(rden[:], den0[:])
                    nc.vector.tensor_scalar_mul(p_r[kt][:, 0:512], e0[:],
                                                rden[:])
                    nc.vector.tensor_scalar_mul(p_r[kt][:, 512:DQ], e1[:],
                                                rden[:])
            pc_cm.__exit__(None, None, None)

            # ------- Phase E: V proj fused with out -----------------------
            with (
                tc.tile_pool(name="pex", bufs=2) as pex,
                tc.tile_pool(name="pev", bufs=2) as pev,
                tc.tile_pool(name="pout", bufs=4) as pout,
                tc.tile_pool(name="pseed", bufs=1) as pseed,
            ):
                seed_sb = pseed.tile([1, 1], f32, tag="seed")
                nc.sync.dma_start(seed_sb[:], seed.ap())
                outv = out.ap().rearrange("(qt p) n -> p qt n", p=P)

                for c in range(NCE):
                    ncol = bass.ds(c * ECH, ECH)
                    if c == 0:
                        xc2 = xe0
                    else:
                        xc2 = pex.tile([P, DT, ECH], f32, tag="xc2",
                                       name=f"xe{c}")
                        nc.sync.dma_start(xc2[:], xv[:, :, ncol])
                    xr2 = pex.tile([P, DT, ECH], f32r, tag="xr2",
                                   name=f"xre{c}")
                    nc.vector.tensor_copy(xr2[:], xc2[:])

                    v_sb = pev.tile([P, KT, ECH], f32r, tag="v", name=f"v{c}")
                    for vt in range(KT):
                        vp = ps.tile([P, ECH], f32, tag="ps",
                                     name=f"vp{c}_{vt}")
                        vsl = bass.ds(vt * P, P)
                        for dt in range(DT):
                            nc.tensor.matmul(
                                vp[:], wv_r[:, dt, vsl], xr2[:, dt],
                                start=(dt == 0), stop=(dt == DT - 1))
                        evict(ei, v_sb[:, vt], vp[:])
                        ei += 1

                    for qg in range(QT128 // 4):
                        osb = pout.tile([P, 4, ECH], f32, tag="osb")
                        for qi in range(4):
                            qt = qg * 4 + qi
                            op = ps.tile([P, ECH], f32, tag="ps",
                                         name=f"op{c}_{qt}")
                            qsl2 = bass.ds(qt * P, P)
                            for kt in range(KT):
                                nc.tensor.matmul(
                                    op[:], p_r[kt][:, qsl2], v_sb[:, kt],
                                    start=(kt == 0), stop=(kt == KT - 1))
                            nc.vector.tensor_copy(osb[:, qi], op[:])
                            if c == 0 and qt == 0:
                                nc.vector.tensor_scalar_add(
                                    osb[0:1, 0, 0:1], op[0:1, 0:1],
                                    seed_sb[:])
                        nc.gpsimd.dma_start(
                            outv[:, qg * 4:(qg + 1) * 4, ncol], osb[:])

            ppr_cm.__exit__(None, None, None)
            pwq0_cm.__exit__(None, None, None)
            pg_cm.__exit__(None, None, None)
            if rep_cm is not None:
                rep_cm.__exit__(None, None, None)
            if sink is not None:
                # touch every out-DMA region (walrus DCE is region-precise):
                # one full row per q-half covers all (qg, c) blocks
                with tc.tile_pool(name="psink", bufs=1) as psink:
                    row0 = psink.tile([1, N], f32, tag="r0", name="row0")
                    row1 = psink.tile([1, N], f32, tag="r1", name="row1")
                    nc.sync.dma_start(row0[:], out.ap()[0:1, :])
                    nc.sync.dma_start(row1[:], out.ap()[DQ // 2:DQ // 2 + 1, :])
                    s0 = psink.tile([1, 1], f32, tag="s0", name="s0")
                    s1 = psink.tile([1, 1], f32, tag="s1", name="s1")
                    nc.vector.reduce_sum(s0[:], row0[:],
                                         axis=mybir.AxisListType.X)
                    nc.vector.reduce_sum(s1[:], row1[:],
                                         axis=mybir.AxisListType.X)
                    nc.vector.tensor_add(s0[:], s0[:], s1[:])
                    nc.sync.dma_start(sink.ap(), s0[:])
            p0_cm.__exit__(None, None, None)

    nc.compile()
    return nc


_CACHE = {}


def _get_nc(DX, N, DQ, DKH):
    key = (DX, N, DQ, DKH)
    if key not in _CACHE:
        _CACHE[key] = _build_core_kernel(DX, N, DQ, DKH)
    return _CACHE[key]


def _run(x, Wq, Wk, Wv, **spmd_kwargs):
    from concourse.bass_utils import run_bass_kernel_spmd

    B, DX, N = x.shape
    DQ = Wq.shape[0]
    DK = Wk.shape[0]
    assert (B, DX, N, DQ, DK) == (B_FULL, DX_FULL, N_FULL, DQ_FULL, DK_FULL)
    DKH = DK // 2

    nc = _get_nc(DX, N, DQ, DKH)

    # Wq/Wk shipped mean-removed (entries - 0.5); the q-varying part of the
    # mean term is restored on-chip via t[q] (see module docstring)
    WqT = np.ascontiguousarray(Wq.T, dtype=np.float32) - np.float32(0.5)
    WkT = np.ascontiguousarray(Wk.T, dtype=np.float32) - np.float32(0.5)
    WvT = np.ascontiguousarray(Wv.T, dtype=np.float32)
    eye = np.eye(128, dtype=np.float32)

    in_maps = []
    for c in range(N_CORES):
        b, h = divmod(c, 2)
        hsl = slice(h * DKH, (h + 1) * DKH)
        in_maps.append({
            "xb": np.ascontiguousarray(x[b], dtype=np.float32),
            "wqt": WqT,
            "wkt": np.ascontiguousarray(WkT[:, hsl]),
            "wvt": np.ascontiguousarray(WvT[:, hsl]),
            "ident": eye,
            "seed": np.zeros((1, 1), np.float32),
        })

    res = run_bass_kernel_spmd(nc, in_maps, core_ids=list(range(N_CORES)),
                               **spmd_kwargs)
    out = np.empty((B, DQ, N), np.float32)
    for b in range(B):
        out[b] = res.results[2 * b]["out"] + res.results[2 * b + 1]["out"]
    return out, res


def kernel(x, Wq, Wk, Wv):
    return _run(x, Wq, Wk, Wv)[0]


# revision 5
# speedup vs baseline: 1.1833x; 1.1833x over previous
"""TRN2 Bass kernel for nn_Attention_369367187796 — Gram-route scores.

Reference (B=4, DX=1024, N=4096, DQ=DK=DV=1024, fp32):
    Q = Wq @ x[b]; K = Wk @ x[b]; V = Wv @ x[b]
    scores = Q @ K.T   (contract n)
    p = softmax(scores / sqrt(DQ), axis=q)   <- softmax over q
    out[q,n] = sum_k p[q,k] V[k,n]

Key algebra: Q and K are used ONLY in scores, and
    scores = Wq (x x^T) Wk^T
so per batch the Q/K/scores path costs 4.3+1.1+1.1 GMAC via the Gram matrix
G = x x^T instead of 12.9 GMAC for Qproj+Kproj+scores. With softmax over q,
any score term constant across q drops out, so mean-removed weights
(Wq' = Wq-0.5, Wk' = Wk-0.5) need only the rank-1 q-varying correction
    t[q] = 0.5 * sum_d Wq'[q,d] g[d],   g = rowsum(G)
restored (two f32r limbs; K-side and const terms cancel in softmax).

Precision (numpy-simulated; sim matches HW for the old direct scheme to
1e-5): single-limb f32r everywhere (G, A=Wq'G, Wk', V, p) gives end-to-end
rel err ~8e-4 vs fp64 (HW-verified) — 25x under the 2e-2 gate.

Sharding: 8 cores = 4 batches x 2 k-halves (DKH=512). Each core computes the
full G/A (duplicated within the pair), its k-half of scores/softmax/V, and
partial out[q,n] summed on the host — no cross-core communication.

Per-core phases:
  A: stream x (256-col chunks), PE-transpose raw f32 -> xT (f32r on evict);
     the first G generation (rows 0-3 x cols 0:512) accumulates in a
     dedicated 4-bank PSUM pool as xT tiles land, keeping PE dense
  B: remaining G generations (32-matmul PSUM chains), mirror lower-left via
     PE-transpose (G symmetric), g = rowsum(G)
  C: A^T[d',q] = G Wq'^T; scoresT[k,q] = Wk'^T_half A^T + ones x (th+tl);
     softmax over q straight from the score PSUMs
  E: stream x again: V k-half projection fused with out = p^T V, DMA out
"""

import math

import numpy as np

B_FULL, DX_FULL, N_FULL = 4, 1024, 4096
DQ_FULL = DK_FULL = 1024
N_CORES = 8


def _build_core_kernel(DX, N, DQ, DKH, bench=False, bench_reps=0):
    import concourse.bass as bass
    import concourse.mybir as mybir
    import concourse.tile as tile
    from concourse import bacc

    f32 = mybir.dt.float32
    f32r = mybir.dt.float32r

    P = 128
    DT = DX // P            # 8 d-tiles (also d' tiles)
    NT = N // P             # 32 n-tiles
    CHA = 256               # phase A chunk cols
    NCA = N // CHA          # 16 chunks
    ECH = 256               # phase E chunk cols
    NCE = N // ECH          # 16 chunks
    KT = DKH // P           # 4 k-tiles
    QT128 = DQ // P         # 8 q-tiles
    QC = DQ // 512          # 2 q-chunks
    scale = 1.0 / math.sqrt(DQ)

    assert DX % P == 0 and N % ECH == 0 and DQ % 512 == 0 and DKH % P == 0

    nc = bacc.Bacc(None, target_bir_lowering=False, debug=False)

    kind_big = "Internal" if bench else "ExternalInput"
    # In bench mode out is Internal (only [1,1] seed/sink cross the tunnel);
    # the post-loop readback of out[0,0] into sink keeps every out write
    # live — without it the compiler dead-code-eliminates most of phase E
    # (observed as a physically impossible 174us/iter).
    kind_out = "Internal" if bench else "ExternalOutput"
    xb = nc.dram_tensor("xb", [DX, N], f32, kind=kind_big)
    wqt = nc.dram_tensor("wqt", [DX, DQ], f32, kind=kind_big)
    wkt = nc.dram_tensor("wkt", [DX, DKH], f32, kind=kind_big)
    wvt = nc.dram_tensor("wvt", [DX, DKH], f32, kind=kind_big)
    # identity for PE transposes: tiny, stays ExternalInput in bench mode too
    ident = nc.dram_tensor("ident", [P, P], f32, kind="ExternalInput")
    seed = nc.dram_tensor("seed", [1, 1], f32, kind="ExternalInput")
    out = nc.dram_tensor("out", [DQ, N], f32, kind=kind_out)
    sink = (nc.dram_tensor("sink", [1, 1], f32, kind="ExternalOutput")
            if bench else None)

    xv = xb.ap().rearrange("(dt p) n -> p dt n", p=P)
    wqv = wqt.ap().rearrange("(dt p) q -> p dt q", p=P)
    wkv = wkt.ap().rearrange("(dt p) k -> p dt k", p=P)
    wvv = wvt.ap().rearrange("(dt p) k -> p dt k", p=P)

    with tile.TileContext(nc) as tc:
        with (
            tc.tile_pool(name="ps", bufs=4, space="PSUM") as ps,
            tc.tile_pool(name="psg", bufs=4, space="PSUM") as psg,
        ):
            p0_cm = tc.tile_pool(name="pres0", bufs=1)
            p0 = p0_cm.__enter__()
            ident_r = p0.tile([P, P], f32r, tag="idr", name="ident_r")
            ident_f = p0.tile([P, P], f32, tag="idf", name="ident_f")
            ones_row = p0.tile([1, P], f32r, tag="ones", name="ones_row")
            g_hi = p0.tile([P, DT, 1], f32r, tag="ghi", name="g_hi")
            g_lo = p0.tile([P, DT, 1], f32r, tag="glo", name="g_lo")

            rep_cm = tc.For_i(0, bench_reps, 1) if bench_reps else None
            if rep_cm is not None:
                rep_cm.__enter__()

            # engine rotation for PSUM->SBUF evictions (GPSIMD can't
            # read PSUM, so alternate DVE and Act)
            def evict(i, dst, src):
                if i % 2 == 0:
                    nc.vector.tensor_copy(dst, src)
                else:
                    nc.scalar.copy(dst, src)

            # prologue: identity + ones (f32r via rounding compute)
            with tc.tile_pool(name="ppro", bufs=1) as ppro:
                istage = ppro.tile([P, P], f32, tag="ist", name="istage")
                nc.sync.dma_start(istage[:], ident.ap())
                nc.sync.dma_start(ident_f[:], ident.ap())
                nc.vector.tensor_copy(ident_r[:], istage[:])
                nc.gpsimd.memset(istage[:, 0:P], 1.0)
                nc.vector.tensor_copy(ones_row[:], istage[0:1, 0:P])

            # ------- Phase A: xT = round(x)^T; G gen0 chases the chunks ----
            pg_cm = tc.tile_pool(name="pg", bufs=1)
            pg = pg_cm.__enter__()
            g_sb = pg.tile([P, DT, DX], f32r, tag="g", name="g_sb")

            # first Wq' quarter prefetched at body start so A-matmuls can
            # begin the moment G completes (the rest of Wq' loads into pc,
            # whose SBUF region only frees when xT dies)
            pwq0_cm = tc.tile_pool(name="pwq0", bufs=1)
            pwq0 = pwq0_cm.__enter__()
            wq_q0 = pwq0.tile([P, DT, 256], f32r, tag="wq0", name="wq_q0")

            pxt_cm = tc.tile_pool(name="pxt", bufs=1)
            pxt = pxt_cm.__enter__()
            xt = pxt.tile([P, NT, DX], f32r, tag="xt", name="xt")

            # gen0: G rows 0-1 x all cols, accumulated as chunks land
            GEN0 = [(0, 0), (0, 1), (1, 0), (1, 1)]   # (row, col-half)
            gp0 = [psg.tile([P, 512], f32, tag="psg", name=f"gp0_{m}")
                   for m in range(4)]

            ei = 0
            with (
                tc.tile_pool(name="pxa", bufs=3) as pxa,
                tc.tile_pool(name="pwq0st", bufs=1) as pwq0st,
            ):
                for c in range(NCA):
                    ncol = bass.ds(c * CHA, CHA)
                    xc = pxa.tile([P, DT, CHA], f32, tag="xc", name=f"xc{c}")
                    nc.sync.dma_start(xc[:], xv[:, :, ncol])
                    if c == 1:
                        w0tmp = pwq0st.tile([P, DT, 256], f32, tag="w0t",
                                            name="w0t")
                        nc.scalar.dma_start(w0tmp[:], wqv[:, :, 0:256])
                        nc.gpsimd.tensor_copy(wq_q0[:], w0tmp[:])
                    for j in range(CHA // P):
                        nt = c * (CHA // P) + j
                        for dh in range(DT // 4):
                            tp = ps.tile([P, 512], f32, tag="ps",
                                         name=f"tp{nt}_{dh}")
                            for di in range(4):
                                dt = dh * 4 + di
                                nc.tensor.transpose(
                                    tp[:, di * P:(di + 1) * P],
                                    xc[:, dt, j * P:(j + 1) * P],
                                    ident_f[:])
                            evict(ei, xt[:, nt, dh * 512:(dh + 1) * 512],
                                  tp[:])
                            ei += 1
                        for m, (row, ch) in enumerate(GEN0):
                            nc.tensor.matmul(
                                gp0[m][:], xt[:, nt, row * P:(row + 1) * P],
                                xt[:, nt, ch * 512:(ch + 1) * 512],
                                start=(nt == 0), stop=(nt == NT - 1))

            # ------- Phase B: remaining G generations, mirror, g ----------
            for m, (row, ch) in enumerate(GEN0):
                evict(ei, g_sb[:, row, ch * 512:(ch + 1) * 512], gp0[m][:])
                ei += 1
            # upper-triangle ragged blocks (row, c0, c1); lower-left comes
            # from the mirror (G symmetric)
            GENS = [(2, 256, 768), (3, 256, 768), (4, 512, 1024),
                    (5, 512, 1024), (2, 768, 1024), (3, 768, 1024),
                    (6, 768, 1024), (7, 768, 1024)]
            for row, c0, c1 in GENS:
                gp = ps.tile([P, c1 - c0], f32, tag="ps",
                             name=f"gp{row}_{c0}")
                for nt in range(NT):
                    nc.tensor.matmul(
                        gp[:], xt[:, nt, row * P:(row + 1) * P],
                        xt[:, nt, bass.ds(c0, c1 - c0)],
                        start=(nt == 0), stop=(nt == NT - 1))
                evict(ei, g_sb[:, row, c0:c1], gp[:])
                ei += 1
            pxt_cm.__exit__(None, None, None)  # free xT (16MB)

            # mirror: G[r, c*128:] = G[c, r*128:]^T for tiles left of each
            # row's directly-computed range
            ROW_START = [0, 0, 2, 2, 4, 4, 6, 6]   # first computed col-tile
            for r in range(2, DT):
                cs = list(range(ROW_START[r]))
                for b0 in range(0, len(cs), 4):
                    grp = cs[b0:b0 + 4]
                    mp = ps.tile([P, len(grp) * P], f32r, tag="ps",
                                 name=f"mp{r}_{b0}")
                    for i, c in enumerate(grp):
                        nc.tensor.transpose(
                            mp[:, i * P:(i + 1) * P],
                            g_sb[:, c, r * P:(r + 1) * P],
                            ident_r[:])
                    evict(ei, g_sb[:, r, grp[0] * P:(grp[-1] + 1) * P],
                          mp[:])
                    ei += 1

            # g = rowsum(G) in two f32r limbs
            with tc.tile_pool(name="pgst", bufs=1) as pgst:
                g_f = pgst.tile([P, DT, 1], f32, tag="gf", name="g_f")
                for dt in range(DT):
                    nc.vector.reduce_sum(g_f[:, dt], g_sb[:, dt],
                                         axis=mybir.AxisListType.X)
                nc.vector.tensor_copy(g_hi[:], g_f[:])
                nc.vector.tensor_sub(g_lo[:], g_f[:], g_hi[:])

            # ------- Phase C: A, t, scores, softmax-from-PSUM -------------
            # ppr: tiles that outlive pc (p_r, wv_r used in E; wv prefetch
            # overlaps C because its buffer can't collide with pc tiles)
            ppr_cm = tc.tile_pool(name="ppr", bufs=1)
            ppr = ppr_cm.__enter__()
            p_r = [ppr.tile([P, DQ], f32r, tag=f"pr{kt}", name=f"p{kt}")
                   for kt in range(KT)]
            wv_r = ppr.tile([P, DT, DKH], f32r, tag="wv", name="wv_r")
            xe0 = ppr.tile([P, DT, ECH], f32, tag="xe0", name="xe0")
            nc.sync.dma_start(xe0[:], xv[:, :, 0:ECH])

            pc_cm = tc.tile_pool(name="pc", bufs=1)
            pc = pc_cm.__enter__()
            wq_r = pc.tile([P, DT, DQ - 256], f32r, tag="wq", name="wq_r")
            wk_r = pc.tile([P, DT, DKH], f32r, tag="wk", name="wk_r")
            a_sb = pc.tile([P, DT, DQ], f32r, tag="a", name="a_sb")
            t_hi = pc.tile([1, DQ], f32r, tag="thi", name="t_hi")
            t_lo = pc.tile([1, DQ], f32r, tag="tlo", name="t_lo")

            with tc.tile_pool(name="pwst", bufs=2) as pwst:
                def wq_ap(dt, q0, q1):
                    # Wq' col range [q0:q1): quarter 0 lives in wq_q0,
                    # the rest in wq_r at offset-256
                    if q1 <= 256:
                        return wq_q0[:, dt, q0:q1]
                    return wq_r[:, dt, q0 - 256:q1 - 256]

                for qq in range(1, DQ // 256):
                    wtmp = pwst.tile([P, DT, 256], f32, tag="wt",
                                     name=f"wt{qq}")
                    (nc.sync, nc.scalar, nc.gpsimd)[qq % 3].dma_start(
                        wtmp[:], wqv[:, :, qq * 256:(qq + 1) * 256])
                    nc.vector.tensor_copy(
                        wq_r[:, :, (qq - 1) * 256:qq * 256], wtmp[:])
                for dt in range(0, DT, 2):
                    d2 = bass.ds(dt, 2)
                    wtmp2 = pwst.tile([P, 2, DKH], f32, tag="wt2",
                                      name=f"wt2{dt}")
                    (nc.scalar, nc.gpsimd)[(dt // 2) % 2].dma_start(
                        wtmp2[:], wkv[:, d2])
                    nc.gpsimd.tensor_copy(wk_r[:, d2], wtmp2[:])
                    wtmp3 = pwst.tile([P, 2, DKH], f32, tag="wt3",
                                      name=f"wt3{dt}")
                    (nc.gpsimd, nc.sync)[(dt // 2) % 2].dma_start(
                        wtmp3[:], wvv[:, d2])
                    nc.gpsimd.tensor_copy(wv_r[:, d2], wtmp3[:])

                # A^T[d', q] = sum_d G[d, d'] Wq'^T[d, q], in 256-wide
                # quarters: quarter 0 uses the prefetched wq_q0 and starts
                # right at G-end, overlapping the wq_r load
                for qq in range(DQ // 256):
                    for dpt in range(DT):
                        dsl = bass.ds(dpt * P, P)
                        ap_ = ps.tile([P, 256], f32, tag="ps",
                                      name=f"ap{dpt}_{qq}")
                        for dt in range(DT):
                            nc.tensor.matmul(
                                ap_[:], g_sb[:, dt, dsl],
                                wq_ap(dt, qq * 256, (qq + 1) * 256),
                                start=(dt == 0), stop=(dt == DT - 1))
                        evict(ei, a_sb[:, dpt, bass.ds(qq * 256, 256)],
                              ap_[:])
                        ei += 1

                # t[q] = 0.5 * (g_hi + g_lo)^T Wq'  (two f32r limbs)
                for qq in range(DQ // 256):
                    qsl = bass.ds(qq * 256, 256)
                    tp2 = ps.tile([P, 256], f32, tag="ps", name=f"tq{qq}")
                    first = True
                    for limb in (g_hi, g_lo):
                        for dt in range(DT):
                            nc.tensor.matmul(
                                tp2[0:1, :], limb[:, dt],
                                wq_ap(dt, qq * 256, (qq + 1) * 256),
                                start=first,
                                stop=(limb is g_lo and dt == DT - 1))
                            first = False
                    nc.scalar.mul(t_hi[:, qsl], tp2[0:1, :], 0.5)
                    nc.vector.scalar_tensor_tensor(
                        t_lo[:, qsl], tp2[0:1, :], 0.5, t_hi[:, qsl],
                        op0=mybir.AluOpType.mult,
                        op1=mybir.AluOpType.subtract)

            with (
                tc.tile_pool(name="psmx", bufs=2) as psmx,
                tc.tile_pool(name="pstat", bufs=4) as pstat,
            ):
                # scoresT[k, q] = Wk'^T A^T + ones x (t_hi + t_lo);
                # softmax over q straight from the two q-chunk PSUMs
                for kt in range(KT):
                    ksl = bass.ds(kt * P, P)
                    sp = []
                    for qc in range(QC):
                        qsl = bass.ds(qc * 512, 512)
                        s = ps.tile([P, 512], f32, tag="ps",
                                    name=f"sp{kt}_{qc}")
                        for dpt in range(DT):
                            nc.tensor.matmul(
                                s[:], wk_r[:, dpt, ksl], a_sb[:, dpt, qsl],
                                start=(dpt == 0), stop=False)
                        nc.tensor.matmul(s[:], ones_row[:], t_hi[:, qsl],
                                         start=False, stop=False)
                        nc.tensor.matmul(s[:], ones_row[:], t_lo[:, qsl],
                                         start=False, stop=True)
                        sp.append(s)
                    m0 = pstat.tile([P, 1], f32, tag="m0")
                    m1 = pstat.tile([P, 1], f32, tag="m1")
                    negm = pstat.tile([P, 1], f32, tag="negm")
                    den0 = pstat.tile([P, 1], f32, tag="den0")
                    den1 = pstat.tile([P, 1], f32, tag="den1")
                    rden = pstat.tile([P, 1], f32, tag="rden")
                    nc.vector.reduce_max(m0[:], sp[0][:],
                                         axis=mybir.AxisListType.X)
                    nc.vector.reduce_max(m1[:], sp[1][:],
                                         axis=mybir.AxisListType.X)
                    nc.vector.tensor_max(m0[:], m0[:], m1[:])
                    nc.vector.tensor_scalar_mul(negm[:], m0[:], -scale)
                    e0 = psmx.tile([P, 512], f32, tag="e0")
                    e1 = psmx.tile([P, 512], f32, tag="e1")
                    nc.scalar.activation(
                        e0[:], sp[0][:], mybir.ActivationFunctionType.Exp,
                        bias=negm[:], scale=scale, accum_out=den0[:])
                    nc.scalar.activation(
                        e1[:], sp[1][:], mybir.ActivationFunctionType.Exp,
                        bias=negm[:], scale=scale, accum_out=den1[:])
                    nc.vector.tensor_add(den0[:], den0[:], den1[:])
                    nc.vector.reciprocal(rden[:], den0[:])
                    nc.vector.tensor_scalar_mul(p_r[kt][:, 0:512], e0[:],
                                                rden[:])
                    nc.vector.tensor_scalar_mul(p_r[kt][:, 512:DQ], e1[:],
                                                rden[:])
            pc_cm.__exit__(None, None, None)

            # ------- Phase E: V proj fused with out -----------------------
            with (
                tc.tile_pool(name="pex", bufs=2) as pex,
                tc.tile_pool(name="pev", bufs=2) as pev,
                tc.tile_pool(name="pout", bufs=4) as pout,
                tc.tile_pool(name="pseed", bufs=1) as pseed,
            ):
                seed_sb = pseed.tile([1, 1], f32, tag="seed")
                nc.sync.dma_start(seed_sb[:], seed.ap())
                outv = out.ap().rearrange("(qt p) n -> p qt n", p=P)

                for c in range(NCE):
                    ncol = bass.ds(c * ECH, ECH)
                    if c == 0:
                        xc2 = xe0
                    else:
                        xc2 = pex.tile([P, DT, ECH], f32, tag="xc2",
                                       name=f"xe{c}")
                        nc.sync.dma_start(xc2[:], xv[:, :, ncol])
                    xr2 = pex.tile([P, DT, ECH], f32r, tag="xr2",
                                   name=f"xre{c}")
                    nc.vector.tensor_copy(xr2[:], xc2[:])

                    v_sb = pev.tile([P, KT, ECH], f32r, tag="v", name=f"v{c}")
                    for vt in range(KT):
                        vp = ps.tile([P, ECH], f32, tag="ps",
                                     name=f"vp{c}_{vt}")
                        vsl = bass.ds(vt * P, P)
                        for dt in range(DT):
                            nc.tensor.matmul(
                                vp[:], wv_r[:, dt, vsl], xr2[:, dt],
                                start=(dt == 0), stop=(dt == DT - 1))
                        evict(ei, v_sb[:, vt], vp[:])
                        ei += 1

                    for qg in range(QT128 // 4):
                        osb = pout.tile([P, 4, ECH], f32, tag="osb")
                        for qi in range(4):
                            qt = qg * 4 + qi
                            op = ps.tile([P, ECH], f32, tag="ps",
                                         name=f"op{c}_{qt}")
                            qsl2 = bass.ds(qt * P, P)
                            for kt in range(KT):
                                nc.tensor.matmul(
                                    op[:], p_r[kt][:, qsl2], v_sb[:, kt],
                                    start=(kt == 0), stop=(kt == KT - 1))
                            nc.vector.tensor_copy(osb[:, qi], op[:])
                            if c == 0 and qt == 0:
                                nc.vector.tensor_scalar_add(
                                    osb[0:1, 0, 0:1], op[0:1, 0:1],
                                    seed_sb[:])
                        nc.gpsimd.dma_start(
                            outv[:, qg * 4:(qg + 1) * 4, ncol], osb[:])

            ppr_cm.__exit__(None, None, None)
            pwq0_cm.__exit__(None, None, None)
            pg_cm.__exit__(None, None, None)
            if rep_cm is not None:
                rep_cm.__exit__(None, None, None)
            if sink is not None:
                # touch every out-DMA region (walrus DCE is region-precise):
                # one full row per q-half covers all (qg, c) blocks
                with tc.tile_pool(name="psink", bufs=1) as psink:
                    row0 = psink.tile([1, N], f32, tag="r0", name="row0")
                    row1 = psink.tile([1, N], f32, tag="r1", name="row1")
                    nc.sync.dma_start(row0[:], out.ap()[0:1, :])
                    nc.sync.dma_start(row1[:], out.ap()[DQ // 2:DQ // 2 + 1, :])
                    s0 = psink.tile([1, 1], f32, tag="s0", name="s0")
                    s1 = psink.tile([1, 1], f32, tag="s1", name="s1")
                    nc.vector.reduce_sum(s0[:], row0[:],
                                         axis=mybir.AxisListType.X)
                    nc.vector.reduce_sum(s1[:], row1[:],
                                         axis=mybir.AxisListType.X)
                    nc.vector.tensor_add(s0[:], s0[:], s1[:])
                    nc.sync.dma_start(sink.ap(), s0[:])
            p0_cm.__exit__(None, None, None)

    nc.compile()
    return nc


_CACHE = {}


def _get_nc(DX, N, DQ, DKH):
    key = (DX, N, DQ, DKH)
    if key not in _CACHE:
        _CACHE[key] = _build_core_kernel(DX, N, DQ, DKH)
    return _CACHE[key]


def _run(x, Wq, Wk, Wv, **spmd_kwargs):
    from concourse.bass_utils import run_bass_kernel_spmd

    B, DX, N = x.shape
    DQ = Wq.shape[0]
    DK = Wk.shape[0]
    assert (B, DX, N, DQ, DK) == (B_FULL, DX_FULL, N_FULL, DQ_FULL, DK_FULL)
    DKH = DK // 2

    nc = _get_nc(DX, N, DQ, DKH)

    # Wq/Wk shipped mean-removed (entries - 0.5); the q-varying part of the
    # mean term is restored on-chip via t[q] (see module docstring)
    WqT = np.ascontiguousarray(Wq.T, dtype=np.float32) - np.float32(0.5)
    WkT = np.ascontiguousarray(Wk.T, dtype=np.float32) - np.float32(0.5)
    WvT = np.ascontiguousarray(Wv.T, dtype=np.float32)
    eye = np.eye(128, dtype=np.float32)

    in_maps = []
    for c in range(N_CORES):
        b, h = divmod(c, 2)
        hsl = slice(h * DKH, (h + 1) * DKH)
        in_maps.append({
            "xb": np.ascontiguousarray(x[b], dtype=np.float32),
            "wqt": WqT,
            "wkt": np.ascontiguousarray(WkT[:, hsl]),
            "wvt": np.ascontiguousarray(WvT[:, hsl]),
            "ident": eye,
            "seed": np.zeros((1, 1), np.float32),
        })

    res = run_bass_kernel_spmd(nc, in_maps, core_ids=list(range(N_CORES)),
                               **spmd_kwargs)
    out = np.empty((B, DQ, N), np.float32)
    for b in range(B):
        out[b] = res.results[2 * b]["out"] + res.results[2 * b + 1]["out"]
    return out, res


def kernel(x, Wq, Wk, Wv):
    return _run(x, Wq, Wk, Wv)[0]


# revision 6
# speedup vs baseline: 1.1847x; 1.0011x over previous
"""TRN2 Bass kernel for nn_Attention_369367187796 — Gram-route scores.

Reference (B=4, DX=1024, N=4096, DQ=DK=DV=1024, fp32):
    Q = Wq @ x[b]; K = Wk @ x[b]; V = Wv @ x[b]
    scores = Q @ K.T   (contract n)
    p = softmax(scores / sqrt(DQ), axis=q)   <- softmax over q
    out[q,n] = sum_k p[q,k] V[k,n]

Key algebra: Q and K are used ONLY in scores, and
    scores = Wq (x x^T) Wk^T
so per batch the Q/K/scores path costs 4.3+1.1+1.1 GMAC via the Gram matrix
G = x x^T instead of 12.9 GMAC for Qproj+Kproj+scores. With softmax over q,
any score term constant across q drops out, so mean-removed weights
(Wq' = Wq-0.5, Wk' = Wk-0.5) need only the rank-1 q-varying correction
    t[q] = 0.5 * sum_d Wq'[q,d] g[d],   g = rowsum(G)
restored (two f32r limbs; K-side and const terms cancel in softmax).

Precision (numpy-simulated; sim matches HW for the old direct scheme to
1e-5): single-limb f32r everywhere (G, A=Wq'G, Wk', V, p) gives end-to-end
rel err ~8e-4 vs fp64 (HW-verified) — 25x under the 2e-2 gate.

Sharding: 8 cores = 4 batches x 2 k-halves (DKH=512). Each core computes the
full G/A (duplicated within the pair), its k-half of scores/softmax/V, and
partial out[q,n] summed on the host — no cross-core communication.

Per-core phases:
  A: stream x (256-col chunks), PE-transpose raw f32 -> xT (f32r on evict);
     the first G generation (rows 0-3 x cols 0:512) accumulates in a
     dedicated 4-bank PSUM pool as xT tiles land, keeping PE dense
  B: remaining G generations (32-matmul PSUM chains), mirror lower-left via
     PE-transpose (G symmetric), g = rowsum(G)
  C: A^T[d',q] = G Wq'^T; scoresT[k,q] = Wk'^T_half A^T + ones x (th+tl);
     softmax over q straight from the score PSUMs
  E: stream x again: V k-half projection fused with out = p^T V, DMA out
"""

import math

import numpy as np

B_FULL, DX_FULL, N_FULL = 4, 1024, 4096
DQ_FULL = DK_FULL = 1024
N_CORES = 8


def _build_core_kernel(DX, N, DQ, DKH, bench=False, bench_reps=0):
    import concourse.bass as bass
    import concourse.mybir as mybir
    import concourse.tile as tile
    from concourse import bacc

    f32 = mybir.dt.float32
    f32r = mybir.dt.float32r
    f8 = mybir.dt.float8e4
    bf16 = mybir.dt.bfloat16

    P = 128
    DT = DX // P            # 8 d-tiles (also d' tiles)
    NT = N // P             # 32 n-tiles
    CHA = 256               # phase A chunk cols
    NCA = N // CHA          # 16 chunks
    ECH = 256               # phase E chunk cols
    NCE = N // ECH          # 16 chunks
    KT = DKH // P           # 4 k-tiles
    QT128 = DQ // P         # 8 q-tiles
    QC = DQ // 512          # 2 q-chunks
    scale = 1.0 / math.sqrt(DQ)

    assert DX % P == 0 and N % ECH == 0 and DQ % 512 == 0 and DKH % P == 0

    nc = bacc.Bacc(None, target_bir_lowering=False, debug=False)

    kind_big = "Internal" if bench else "ExternalInput"
    # In bench mode out is Internal (only [1,1] seed/sink cross the tunnel);
    # the post-loop readback of out[0,0] into sink keeps every out write
    # live — without it the compiler dead-code-eliminates most of phase E
    # (observed as a physically impossible 174us/iter).
    kind_out = "Internal" if bench else "ExternalOutput"
    xb = nc.dram_tensor("xb", [DX, N], f32, kind=kind_big)
    wqt = nc.dram_tensor("wqt", [DX, DQ], f32, kind=kind_big)
    wkt = nc.dram_tensor("wkt", [DX, DKH], f32, kind=kind_big)
    wvt = nc.dram_tensor("wvt", [DX, DKH], f32, kind=kind_big)
    # identity for PE transposes: tiny, stays ExternalInput in bench mode too
    ident = nc.dram_tensor("ident", [P, P], f32, kind="ExternalInput")
    seed = nc.dram_tensor("seed", [1, 1], f32, kind="ExternalInput")
    # out in bf16: halves the 16MB output write (phase E is DMA-bound
    # after the fp8 out-matmul); host upcasts to f32. Adds ~2e-3 rel err.
    out = nc.dram_tensor("out", [DQ, N], bf16, kind=kind_out)
    sink = (nc.dram_tensor("sink", [1, 1], f32, kind="ExternalOutput")
            if bench else None)

    xv = xb.ap().rearrange("(dt p) n -> p dt n", p=P)
    wqv = wqt.ap().rearrange("(dt p) q -> p dt q", p=P)
    wkv = wkt.ap().rearrange("(dt p) k -> p dt k", p=P)
    wvv = wvt.ap().rearrange("(dt p) k -> p dt k", p=P)

    with tile.TileContext(nc) as tc:
        with (
            tc.tile_pool(name="ps", bufs=4, space="PSUM") as ps,
            tc.tile_pool(name="psg", bufs=4, space="PSUM") as psg,
        ):
            p0_cm = tc.tile_pool(name="pres0", bufs=1)
            p0 = p0_cm.__enter__()
            ident_r = p0.tile([P, P], f32r, tag="idr", name="ident_r")
            ident_f = p0.tile([P, P], f32, tag="idf", name="ident_f")
            ones_row = p0.tile([1, P], f32r, tag="ones", name="ones_row")
            g_hi = p0.tile([P, DT, 1], f32r, tag="ghi", name="g_hi")
            g_lo = p0.tile([P, DT, 1], f32r, tag="glo", name="g_lo")

            rep_cm = tc.For_i(0, bench_reps, 1) if bench_reps else None
            if rep_cm is not None:
                rep_cm.__enter__()

            # engine rotation for PSUM->SBUF evictions (GPSIMD can't
            # read PSUM, so alternate DVE and Act)
            def evict(i, dst, src):
                if i % 2 == 0:
                    nc.vector.tensor_copy(dst, src)
                else:
                    nc.scalar.copy(dst, src)

            # prologue: identity + ones (f32r via rounding compute)
            with tc.tile_pool(name="ppro", bufs=1) as ppro:
                istage = ppro.tile([P, P], f32, tag="ist", name="istage")
                nc.sync.dma_start(istage[:], ident.ap())
                nc.sync.dma_start(ident_f[:], ident.ap())
                nc.vector.tensor_copy(ident_r[:], istage[:])
                nc.gpsimd.memset(istage[:, 0:P], 1.0)
                nc.vector.tensor_copy(ones_row[:], istage[0:1, 0:P])

            # ------- Phase A: xT = round(x)^T; G gen0 chases the chunks ----
            pg_cm = tc.tile_pool(name="pg", bufs=1)
            pg = pg_cm.__enter__()
            g_sb = pg.tile([P, DT, DX], f32r, tag="g", name="g_sb")

            # first Wq' quarter prefetched at body start so A-matmuls can
            # begin the moment G completes (the rest of Wq' loads into pc,
            # whose SBUF region only frees when xT dies)
            pwq0_cm = tc.tile_pool(name="pwq0", bufs=1)
            pwq0 = pwq0_cm.__enter__()
            wq_q0 = pwq0.tile([P, DT, 256], f32r, tag="wq0", name="wq_q0")

            pxt_cm = tc.tile_pool(name="pxt", bufs=1)
            pxt = pxt_cm.__enter__()
            xt = pxt.tile([P, NT, DX], f32r, tag="xt", name="xt")

            # gen0: G rows 0-1 x all cols, accumulated as chunks land
            GEN0 = [(0, 0), (0, 1), (1, 0), (1, 1)]   # (row, col-half)
            gp0 = [psg.tile([P, 512], f32, tag="psg", name=f"gp0_{m}")
                   for m in range(4)]

            ei = 0
            with (
                tc.tile_pool(name="pxa", bufs=3) as pxa,
                tc.tile_pool(name="pwq0st", bufs=1) as pwq0st,
            ):
                for c in range(NCA):
                    ncol = bass.ds(c * CHA, CHA)
                    xc = pxa.tile([P, DT, CHA], f32, tag="xc", name=f"xc{c}")
                    nc.sync.dma_start(xc[:], xv[:, :, ncol])
                    if c == 1:
                        w0tmp = pwq0st.tile([P, DT, 256], f32, tag="w0t",
                                            name="w0t")
                        nc.scalar.dma_start(w0tmp[:], wqv[:, :, 0:256])
                        nc.gpsimd.tensor_copy(wq_q0[:], w0tmp[:])
                    for j in range(CHA // P):
                        nt = c * (CHA // P) + j
                        for dh in range(DT // 4):
                            tp = ps.tile([P, 512], f32, tag="ps",
                                         name=f"tp{nt}_{dh}")
                            for di in range(4):
                                dt = dh * 4 + di
                                nc.tensor.transpose(
                                    tp[:, di * P:(di + 1) * P],
                                    xc[:, dt, j * P:(j + 1) * P],
                                    ident_f[:])
                            evict(ei, xt[:, nt, dh * 512:(dh + 1) * 512],
                                  tp[:])
                            ei += 1
                        for m, (row, ch) in enumerate(GEN0):
                            nc.tensor.matmul(
                                gp0[m][:], xt[:, nt, row * P:(row + 1) * P],
                                xt[:, nt, ch * 512:(ch + 1) * 512],
                                start=(nt == 0), stop=(nt == NT - 1))

            # ------- Phase B: remaining G generations, mirror, g ----------
            for m, (row, ch) in enumerate(GEN0):
                evict(ei, g_sb[:, row, ch * 512:(ch + 1) * 512], gp0[m][:])
                ei += 1
            # upper-triangle ragged blocks (row, c0, c1); lower-left comes
            # from the mirror (G symmetric)
            GENS = [(2, 256, 768), (3, 256, 768), (4, 512, 1024),
                    (5, 512, 1024), (2, 768, 1024), (3, 768, 1024),
                    (6, 768, 1024), (7, 768, 1024)]
            for row, c0, c1 in GENS:
                gp = ps.tile([P, c1 - c0], f32, tag="ps",
                             name=f"gp{row}_{c0}")
                for nt in range(NT):
                    nc.tensor.matmul(
                        gp[:], xt[:, nt, row * P:(row + 1) * P],
                        xt[:, nt, bass.ds(c0, c1 - c0)],
                        start=(nt == 0), stop=(nt == NT - 1))
                evict(ei, g_sb[:, row, c0:c1], gp[:])
                ei += 1
            pxt_cm.__exit__(None, None, None)  # free xT (16MB)

            # mirror: G[r, c*128:] = G[c, r*128:]^T for tiles left of each
            # row's directly-computed range
            ROW_START = [0, 0, 2, 2, 4, 4, 6, 6]   # first computed col-tile
            for r in range(2, DT):
                cs = list(range(ROW_START[r]))
                for b0 in range(0, len(cs), 4):
                    grp = cs[b0:b0 + 4]
                    mp = ps.tile([P, len(grp) * P], f32r, tag="ps",
                                 name=f"mp{r}_{b0}")
                    for i, c in enumerate(grp):
                        nc.tensor.transpose(
                            mp[:, i * P:(i + 1) * P],
                            g_sb[:, c, r * P:(r + 1) * P],
                            ident_r[:])
                    evict(ei, g_sb[:, r, grp[0] * P:(grp[-1] + 1) * P],
                          mp[:])
                    ei += 1

            # g = rowsum(G) in two f32r limbs
            with tc.tile_pool(name="pgst", bufs=1) as pgst:
                g_f = pgst.tile([P, DT, 1], f32, tag="gf", name="g_f")
                for dt in range(DT):
                    nc.vector.reduce_sum(g_f[:, dt], g_sb[:, dt],
                                         axis=mybir.AxisListType.X)
                nc.vector.tensor_copy(g_hi[:], g_f[:])
                nc.vector.tensor_sub(g_lo[:], g_f[:], g_hi[:])

            # ------- Phase C: A, t, scores, softmax-from-PSUM -------------
            # ppr: tiles that outlive pc (p_r, wv_r used in E; wv prefetch
            # overlaps C because its buffer can't collide with pc tiles)
            ppr_cm = tc.tile_pool(name="ppr", bufs=1)
            ppr = ppr_cm.__enter__()
            p_r8 = ppr.tile([P, KT, DQ], f8, tag="pr8", name="p_r8")
            wv_r = ppr.tile([P, DT, DKH], f32r, tag="wv", name="wv_r")
            xe0 = ppr.tile([P, DT, ECH], f32, tag="xe0", name="xe0")
            nc.sync.dma_start(xe0[:], xv[:, :, 0:ECH])

            pc_cm = tc.tile_pool(name="pc", bufs=1)
            pc = pc_cm.__enter__()
            wq_r = pc.tile([P, DT, DQ - 256], f32r, tag="wq", name="wq_r")
            wk_r = pc.tile([P, DT, DKH], f32r, tag="wk", name="wk_r")
            a_sb = pc.tile([P, DT, DQ], f32r, tag="a", name="a_sb")
            t_hi = pc.tile([1, DQ], f32r, tag="thi", name="t_hi")
            t_lo = pc.tile([1, DQ], f32r, tag="tlo", name="t_lo")

            with tc.tile_pool(name="pwst", bufs=2) as pwst:
                def wq_ap(dt, q0, q1):
                    # Wq' col range [q0:q1): quarter 0 lives in wq_q0,
                    # the rest in wq_r at offset-256
                    if q1 <= 256:
                        return wq_q0[:, dt, q0:q1]
                    return wq_r[:, dt, q0 - 256:q1 - 256]

                for qq in range(1, DQ // 256):
                    wtmp = pwst.tile([P, DT, 256], f32, tag="wt",
                                     name=f"wt{qq}")
                    (nc.sync, nc.scalar, nc.gpsimd)[qq % 3].dma_start(
                        wtmp[:], wqv[:, :, qq * 256:(qq + 1) * 256])
                    nc.vector.tensor_copy(
                        wq_r[:, :, (qq - 1) * 256:qq * 256], wtmp[:])
                for dt in range(0, DT, 2):
                    d2 = bass.ds(dt, 2)
                    wtmp2 = pwst.tile([P, 2, DKH], f32, tag="wt2",
                                      name=f"wt2{dt}")
                    (nc.scalar, nc.gpsimd)[(dt // 2) % 2].dma_start(
                        wtmp2[:], wkv[:, d2])
                    nc.gpsimd.tensor_copy(wk_r[:, d2], wtmp2[:])
                    wtmp3 = pwst.tile([P, 2, DKH], f32, tag="wt3",
                                      name=f"wt3{dt}")
                    (nc.gpsimd, nc.sync)[(dt // 2) % 2].dma_start(
                        wtmp3[:], wvv[:, d2])
                    nc.gpsimd.tensor_copy(wv_r[:, d2], wtmp3[:])

                # A^T[d', q] = sum_d G[d, d'] Wq'^T[d, q], in 256-wide
                # quarters: quarter 0 uses the prefetched wq_q0 and starts
                # right at G-end, overlapping the wq_r load
                for qq in range(DQ // 256):
                    for dpt in range(DT):
                        dsl = bass.ds(dpt * P, P)
                        ap_ = ps.tile([P, 256], f32, tag="ps",
                                      name=f"ap{dpt}_{qq}")
                        for dt in range(DT):
                            nc.tensor.matmul(
                                ap_[:], g_sb[:, dt, dsl],
                                wq_ap(dt, qq * 256, (qq + 1) * 256),
                                start=(dt == 0), stop=(dt == DT - 1))
                        evict(ei, a_sb[:, dpt, bass.ds(qq * 256, 256)],
                              ap_[:])
                        ei += 1

                # t[q] = 0.5 * (g_hi + g_lo)^T Wq'  (two f32r limbs)
                for qq in range(DQ // 256):
                    qsl = bass.ds(qq * 256, 256)
                    tp2 = ps.tile([P, 256], f32, tag="ps", name=f"tq{qq}")
                    first = True
                    for limb in (g_hi, g_lo):
                        for dt in range(DT):
                            nc.tensor.matmul(
                                tp2[0:1, :], limb[:, dt],
                                wq_ap(dt, qq * 256, (qq + 1) * 256),
                                start=first,
                                stop=(limb is g_lo and dt == DT - 1))
                            first = False
                    nc.scalar.mul(t_hi[:, qsl], tp2[0:1, :], 0.5)
                    nc.vector.scalar_tensor_tensor(
                        t_lo[:, qsl], tp2[0:1, :], 0.5, t_hi[:, qsl],
                        op0=mybir.AluOpType.mult,
                        op1=mybir.AluOpType.subtract)

            with (
                tc.tile_pool(name="psmx", bufs=2) as psmx,
                tc.tile_pool(name="pstat", bufs=4) as pstat,
            ):
                # scoresT[k, q] = Wk'^T A^T + ones x (t_hi + t_lo);
                # softmax over q straight from the two q-chunk PSUMs
                for kt in range(KT):
                    ksl = bass.ds(kt * P, P)
                    sp = []
                    for qc in range(QC):
                        qsl = bass.ds(qc * 512, 512)
                        s = ps.tile([P, 512], f32, tag="ps",
                                    name=f"sp{kt}_{qc}")
                        for dpt in range(DT):
                            nc.tensor.matmul(
                                s[:], wk_r[:, dpt, ksl], a_sb[:, dpt, qsl],
                                start=(dpt == 0), stop=False)
                        nc.tensor.matmul(s[:], ones_row[:], t_hi[:, qsl],
                                         start=False, stop=False)
                        nc.tensor.matmul(s[:], ones_row[:], t_lo[:, qsl],
                                         start=False, stop=True)
                        sp.append(s)
                    m0 = pstat.tile([P, 1], f32, tag="m0")
                    m1 = pstat.tile([P, 1], f32, tag="m1")
                    negm = pstat.tile([P, 1], f32, tag="negm")
                    den0 = pstat.tile([P, 1], f32, tag="den0")
                    den1 = pstat.tile([P, 1], f32, tag="den1")
                    rden = pstat.tile([P, 1], f32, tag="rden")
                    nc.vector.reduce_max(m0[:], sp[0][:],
                                         axis=mybir.AxisListType.X)
                    nc.vector.reduce_max(m1[:], sp[1][:],
                                         axis=mybir.AxisListType.X)
                    nc.vector.tensor_max(m0[:], m0[:], m1[:])
                    nc.vector.tensor_scalar_mul(negm[:], m0[:], -scale)
                    e0 = psmx.tile([P, 512], f32, tag="e0")
                    e1 = psmx.tile([P, 512], f32, tag="e1")
                    nc.scalar.activation(
                        e0[:], sp[0][:], mybir.ActivationFunctionType.Exp,
                        bias=negm[:], scale=scale, accum_out=den0[:])
                    nc.scalar.activation(
                        e1[:], sp[1][:], mybir.ActivationFunctionType.Exp,
                        bias=negm[:], scale=scale, accum_out=den1[:])
                    nc.vector.tensor_add(den0[:], den0[:], den1[:])
                    nc.vector.reciprocal(rden[:], den0[:])
                    nc.vector.tensor_scalar_mul(p_r8[:, kt, 0:512], e0[:],
                                                rden[:])
                    nc.vector.tensor_scalar_mul(p_r8[:, kt, 512:DQ], e1[:],
                                                rden[:])
            pc_cm.__exit__(None, None, None)

            # ------- Phase E: V proj fused with out -----------------------
            with (
                tc.tile_pool(name="pex", bufs=2) as pex,
                tc.tile_pool(name="pev", bufs=2) as pev,
                tc.tile_pool(name="pout", bufs=4) as pout,
                tc.tile_pool(name="pseed", bufs=1) as pseed,
            ):
                seed_sb = pseed.tile([1, 1], f32, tag="seed")
                nc.sync.dma_start(seed_sb[:], seed.ap())
                outv = out.ap().rearrange("(qt p) n -> p qt n", p=P)

                for c in range(NCE):
                    ncol = bass.ds(c * ECH, ECH)
                    if c == 0:
                        xc2 = xe0
                    else:
                        xc2 = pex.tile([P, DT, ECH], f32, tag="xc2",
                                       name=f"xe{c}")
                        nc.sync.dma_start(xc2[:], xv[:, :, ncol])
                    xr2 = pex.tile([P, DT, ECH], f32r, tag="xr2",
                                   name=f"xre{c}")
                    nc.scalar.copy(xr2[:], xc2[:])

                    v_sb = pev.tile([P, KT, ECH], f8, tag="v", name=f"v{c}")
                    for vt in range(KT):
                        vp = ps.tile([P, ECH], f32, tag="ps",
                                     name=f"vp{c}_{vt}")
                        vsl = bass.ds(vt * P, P)
                        for dt in range(DT):
                            nc.tensor.matmul(
                                vp[:], wv_r[:, dt, vsl], xr2[:, dt],
                                start=(dt == 0), stop=(dt == DT - 1))
                        evict(ei, v_sb[:, vt], vp[:])
                        ei += 1

                    for qg in range(QT128 // 4):
                        osb = pout.tile([P, 4, ECH], bf16, tag="osb")
                        for qi in range(4):
                            qt = qg * 4 + qi
                            op = ps.tile([P, ECH], f32, tag="ps",
                                         name=f"op{c}_{qt}")
                            qsl2 = bass.ds(qt * P, P)
                            for kp in range(KT // 2):
                                nc.tensor.matmul(
                                    op[:],
                                    p_r8[:, 2 * kp:2 * kp + 2, qsl2],
                                    v_sb[:, 2 * kp:2 * kp + 2, :],
                                    start=(kp == 0), stop=(kp == KT // 2 - 1),
                                    perf_mode=mybir.MatmulPerfMode.DoubleRow)
                            nc.vector.tensor_copy(osb[:, qi], op[:])
                            if c == 0 and qt == 0:
                                nc.vector.tensor_scalar_add(
                                    osb[0:1, 0, 0:1], op[0:1, 0:1],
                                    seed_sb[:])
                        nc.sync.dma_start(
                            outv[:, qg * 4:(qg + 1) * 4, ncol], osb[:])

            ppr_cm.__exit__(None, None, None)
            pwq0_cm.__exit__(None, None, None)
            pg_cm.__exit__(None, None, None)
            if rep_cm is not None:
                rep_cm.__exit__(None, None, None)
            if sink is not None:
                # touch every out-DMA region (walrus DCE is region-precise):
                # one full row per q-half covers all (qg, c) blocks
                with tc.tile_pool(name="psink", bufs=1) as psink:
                    row0 = psink.tile([1, N], bf16, tag="r0", name="row0")
                    row1 = psink.tile([1, N], bf16, tag="r1", name="row1")
                    nc.sync.dma_start(row0[:], out.ap()[0:1, :])
                    nc.sync.dma_start(row1[:], out.ap()[DQ // 2:DQ // 2 + 1, :])
                    s0 = psink.tile([1, 1], f32, tag="s0", name="s0")
                    s1 = psink.tile([1, 1], f32, tag="s1", name="s1")
                    nc.vector.reduce_sum(s0[:], row0[:],
                                         axis=mybir.AxisListType.X)
                    nc.vector.reduce_sum(s1[:], row1[:],
                                         axis=mybir.AxisListType.X)
                    nc.vector.tensor_add(s0[:], s0[:], s1[:])
                    nc.sync.dma_start(sink.ap(), s0[:])
            p0_cm.__exit__(None, None, None)

    nc.compile()
    return nc


_CACHE = {}


def _get_nc(DX, N, DQ, DKH):
    key = (DX, N, DQ, DKH)
    if key not in _CACHE:
        _CACHE[key] = _build_core_kernel(DX, N, DQ, DKH)
    return _CACHE[key]


def _run(x, Wq, Wk, Wv, **spmd_kwargs):
    from concourse.bass_utils import run_bass_kernel_spmd

    B, DX, N = x.shape
    DQ = Wq.shape[0]
    DK = Wk.shape[0]
    assert (B, DX, N, DQ, DK) == (B_FULL, DX_FULL, N_FULL, DQ_FULL, DK_FULL)
    DKH = DK // 2

    nc = _get_nc(DX, N, DQ, DKH)

    # Wq/Wk shipped mean-removed (entries - 0.5); the q-varying part of the
    # mean term is restored on-chip via t[q] (see module docstring)
    WqT = np.ascontiguousarray(Wq.T, dtype=np.float32) - np.float32(0.5)
    WkT = np.ascontiguousarray(Wk.T, dtype=np.float32) - np.float32(0.5)
    WvT = np.ascontiguousarray(Wv.T, dtype=np.float32)
    eye = np.eye(128, dtype=np.float32)

    in_maps = []
    for c in range(N_CORES):
        b, h = divmod(c, 2)
        hsl = slice(h * DKH, (h + 1) * DKH)
        in_maps.append({
            "xb": np.ascontiguousarray(x[b], dtype=np.float32),
            "wqt": WqT,
            "wkt": np.ascontiguousarray(WkT[:, hsl]),
            "wvt": np.ascontiguousarray(WvT[:, hsl]),
            "ident": eye,
            "seed": np.zeros((1, 1), np.float32),
        })

    res = run_bass_kernel_spmd(nc, in_maps, core_ids=list(range(N_CORES)),
                               **spmd_kwargs)
    out = np.empty((B, DQ, N), np.float32)
    for b in range(B):
        out[b] = (np.asarray(res.results[2 * b]["out"]).astype(np.float32)
                  + np.asarray(res.results[2 * b + 1]["out"]).astype(
                      np.float32))
    return out, res


def kernel(x, Wq, Wk, Wv):
    return _run(x, Wq, Wk, Wv)[0]


# revision 7
# speedup vs baseline: 1.3242x; 1.1178x over previous
"""TRN2 Bass kernel for nn_Attention_369367187796 — Gram-route scores.

Reference (B=4, DX=1024, N=4096, DQ=DK=DV=1024, fp32):
    Q = Wq @ x[b]; K = Wk @ x[b]; V = Wv @ x[b]
    scores = Q @ K.T   (contract n)
    p = softmax(scores / sqrt(DQ), axis=q)   <- softmax over q
    out[q,n] = sum_k p[q,k] V[k,n]

Key algebra: Q and K are used ONLY in scores, and
    scores = Wq (x x^T) Wk^T
so per batch the Q/K/scores path costs 4.3+1.1+1.1 GMAC via the Gram matrix
G = x x^T instead of 12.9 GMAC for Qproj+Kproj+scores. With softmax over q,
any score term constant across q drops out, so mean-removed weights
(Wq' = Wq-0.5, Wk' = Wk-0.5) need only the rank-1 q-varying correction
    t[q] = 0.5 * sum_d Wq'[q,d] g[d],   g = rowsum(G)
restored (two f32r limbs; K-side and const terms cancel in softmax).

Precision (numpy-simulated; sim matches HW for the old direct scheme to
1e-5): single-limb f32r everywhere (G, A=Wq'G, Wk', V, p) gives end-to-end
rel err ~8e-4 vs fp64 (HW-verified) — 25x under the 2e-2 gate.

Sharding: 8 cores = 4 batches x 2 k-halves (DKH=512). Each core computes the
full G/A (duplicated within the pair), its k-half of scores/softmax/V, and
partial out[q,n] summed on the host — no cross-core communication.

Per-core phases:
  A: stream x (256-col chunks), PE-transpose raw f32 -> xT (f32r on evict);
     the first G generation (rows 0-3 x cols 0:512) accumulates in a
     dedicated 4-bank PSUM pool as xT tiles land, keeping PE dense
  B: remaining G generations (32-matmul PSUM chains), mirror lower-left via
     PE-transpose (G symmetric), g = rowsum(G)
  C: A^T[d',q] = G Wq'^T; scoresT[k,q] = Wk'^T_half A^T + ones x (th+tl);
     softmax over q straight from the score PSUMs
  E: stream x again: V k-half projection fused with out = p^T V, DMA out
"""

import math

import numpy as np

B_FULL, DX_FULL, N_FULL = 4, 1024, 4096
DQ_FULL = DK_FULL = 1024
N_CORES = 8


def _build_core_kernel(DX, N, DQ, DKH, bench=False, bench_reps=0):
    import concourse.bass as bass
    import concourse.mybir as mybir
    import concourse.tile as tile
    from concourse import bacc

    f32 = mybir.dt.float32
    f32r = mybir.dt.float32r
    f8 = mybir.dt.float8e4
    bf16 = mybir.dt.bfloat16

    P = 128
    DT = DX // P            # 8 d-tiles (also d' tiles)
    NT = N // P             # 32 n-tiles
    CHA = 256               # phase A chunk cols
    NCA = N // CHA          # 16 chunks
    ECH = 256               # phase E chunk cols
    NCE = N // ECH          # 16 chunks
    KT = DKH // P           # 4 k-tiles
    QT128 = DQ // P         # 8 q-tiles
    QC = DQ // 512          # 2 q-chunks
    scale = 1.0 / math.sqrt(DQ)

    assert DX % P == 0 and N % ECH == 0 and DQ % 512 == 0 and DKH % P == 0

    nc = bacc.Bacc(None, target_bir_lowering=False, debug=False)

    kind_big = "Internal" if bench else "ExternalInput"
    # In bench mode out is Internal (only [1,1] seed/sink cross the tunnel);
    # the post-loop readback of out[0,0] into sink keeps every out write
    # live — without it the compiler dead-code-eliminates most of phase E
    # (observed as a physically impossible 174us/iter).
    kind_out = "Internal" if bench else "ExternalOutput"
    xb = nc.dram_tensor("xb", [DX, N], f32, kind=kind_big)
    wqt = nc.dram_tensor("wqt", [DX, DQ], f32, kind=kind_big)
    wkt = nc.dram_tensor("wkt", [DX, DKH], f32, kind=kind_big)
    wvt = nc.dram_tensor("wvt", [DX, DKH], f32, kind=kind_big)
    # identity for PE transposes: tiny, stays ExternalInput in bench mode too
    ident = nc.dram_tensor("ident", [P, P], f32, kind="ExternalInput")
    seed = nc.dram_tensor("seed", [1, 1], f32, kind="ExternalInput")
    # out in bf16: halves the 16MB output write (phase E is DMA-bound
    # after the fp8 out-matmul); host upcasts to f32. Adds ~2e-3 rel err.
    out = nc.dram_tensor("out", [DQ, N], bf16, kind=kind_out)
    sink = (nc.dram_tensor("sink", [1, 1], f32, kind="ExternalOutput")
            if bench else None)

    xv = xb.ap().rearrange("(dt p) n -> p dt n", p=P)
    wqv = wqt.ap().rearrange("(dt p) q -> p dt q", p=P)
    wkv = wkt.ap().rearrange("(dt p) k -> p dt k", p=P)
    wvv = wvt.ap().rearrange("(dt p) k -> p dt k", p=P)

    with tile.TileContext(nc) as tc:
        with (
            tc.tile_pool(name="ps", bufs=4, space="PSUM") as ps,
            tc.tile_pool(name="psg", bufs=4, space="PSUM") as psg,
        ):
            p0_cm = tc.tile_pool(name="pres0", bufs=1)
            p0 = p0_cm.__enter__()
            ident_r = p0.tile([P, P], f32r, tag="idr", name="ident_r")
            ident_f = p0.tile([P, P], f32, tag="idf", name="ident_f")
            ones_row = p0.tile([1, P], f32r, tag="ones", name="ones_row")
            g_hi = p0.tile([P, DT, 1], f32r, tag="ghi", name="g_hi")
            g_lo = p0.tile([P, DT, 1], f32r, tag="glo", name="g_lo")

            rep_cm = tc.For_i(0, bench_reps, 1) if bench_reps else None
            if rep_cm is not None:
                rep_cm.__enter__()

            # engine rotation for PSUM->SBUF evictions (GPSIMD can't
            # read PSUM, so alternate DVE and Act)
            def evict(i, dst, src):
                if i % 2 == 0:
                    nc.vector.tensor_copy(dst, src)
                else:
                    nc.scalar.copy(dst, src)

            # prologue: identity + ones (f32r via rounding compute)
            with tc.tile_pool(name="ppro", bufs=1) as ppro:
                istage = ppro.tile([P, P], f32, tag="ist", name="istage")
                nc.sync.dma_start(istage[:], ident.ap())
                nc.sync.dma_start(ident_f[:], ident.ap())
                nc.vector.tensor_copy(ident_r[:], istage[:])
                nc.gpsimd.memset(istage[:, 0:P], 1.0)
                nc.vector.tensor_copy(ones_row[:], istage[0:1, 0:P])

            # ------- Phase A: xT = round(x)^T; G gen0 chases the chunks ----
            pg_cm = tc.tile_pool(name="pg", bufs=1)
            pg = pg_cm.__enter__()
            g_sb = pg.tile([P, DT, DX], f32r, tag="g", name="g_sb")

            # first Wq' quarter prefetched at body start so A-matmuls can
            # begin the moment G completes (the rest of Wq' loads into pc,
            # whose SBUF region only frees when xT dies)
            pwq0_cm = tc.tile_pool(name="pwq0", bufs=1)
            pwq0 = pwq0_cm.__enter__()
            wq_q0 = pwq0.tile([P, DT, 256], f32r, tag="wq0", name="wq_q0")

            pxt_cm = tc.tile_pool(name="pxt", bufs=1)
            pxt = pxt_cm.__enter__()
            xt = pxt.tile([P, NT, DX], f32r, tag="xt", name="xt")

            # gen0: G rows 0-1 x all cols, accumulated as chunks land
            GEN0 = [(0, 0), (0, 1), (1, 0), (1, 1)]   # (row, col-half)
            gp0 = [psg.tile([P, 512], f32, tag="psg", name=f"gp0_{m}")
                   for m in range(4)]

            ei = 0
            with (
                tc.tile_pool(name="pxa", bufs=3) as pxa,
                tc.tile_pool(name="pwq0st", bufs=1) as pwq0st,
            ):
                for c in range(NCA):
                    ncol = bass.ds(c * CHA, CHA)
                    xc = pxa.tile([P, DT, CHA], f32, tag="xc", name=f"xc{c}")
                    if c == 0:
                        nc.sync.dma_start(xc[:, :, 0:CHA // 2],
                                          xv[:, :, 0:CHA // 2])
                        nc.sync.dma_start(xc[:, :, CHA // 2:CHA],
                                          xv[:, :, CHA // 2:CHA])
                    else:
                        nc.sync.dma_start(xc[:], xv[:, :, ncol])
                    if c == 1:
                        w0tmp = pwq0st.tile([P, DT, 256], f32, tag="w0t",
                                            name="w0t")
                        nc.scalar.dma_start(w0tmp[:], wqv[:, :, 0:256])
                        nc.gpsimd.tensor_copy(wq_q0[:], w0tmp[:])
                    for j in range(CHA // P):
                        nt = c * (CHA // P) + j
                        for dh in range(DT // 4):
                            tp = ps.tile([P, 512], f32, tag="ps",
                                         name=f"tp{nt}_{dh}")
                            for di in range(4):
                                dt = dh * 4 + di
                                nc.tensor.transpose(
                                    tp[:, di * P:(di + 1) * P],
                                    xc[:, dt, j * P:(j + 1) * P],
                                    ident_f[:])
                            evict(ei, xt[:, nt, dh * 512:(dh + 1) * 512],
                                  tp[:])
                            ei += 1
                        for m, (row, ch) in enumerate(GEN0):
                            nc.tensor.matmul(
                                gp0[m][:], xt[:, nt, row * P:(row + 1) * P],
                                xt[:, nt, ch * 512:(ch + 1) * 512],
                                start=(nt == 0), stop=(nt == NT - 1))

            # ------- Phase B: remaining G generations, mirror, g ----------
            for m, (row, ch) in enumerate(GEN0):
                evict(ei, g_sb[:, row, ch * 512:(ch + 1) * 512], gp0[m][:])
                ei += 1
            # upper-triangle ragged blocks (row, c0, c1); lower-left comes
            # from the mirror (G symmetric)
            GENS = [(2, 256, 768), (3, 256, 768), (4, 512, 1024),
                    (5, 512, 1024), (2, 768, 1024), (3, 768, 1024),
                    (6, 768, 1024), (7, 768, 1024)]
            for row, c0, c1 in GENS:
                gp = ps.tile([P, c1 - c0], f32, tag="ps",
                             name=f"gp{row}_{c0}")
                for nt in range(NT):
                    nc.tensor.matmul(
                        gp[:], xt[:, nt, row * P:(row + 1) * P],
                        xt[:, nt, bass.ds(c0, c1 - c0)],
                        start=(nt == 0), stop=(nt == NT - 1))
                evict(ei, g_sb[:, row, c0:c1], gp[:])
                ei += 1
            pxt_cm.__exit__(None, None, None)  # free xT (16MB)

            # mirror: G[r, c*128:] = G[c, r*128:]^T for tiles left of each
            # row's directly-computed range
            ROW_START = [0, 0, 2, 2, 4, 4, 6, 6]   # first computed col-tile
            for r in range(2, DT):
                cs = list(range(ROW_START[r]))
                for b0 in range(0, len(cs), 4):
                    grp = cs[b0:b0 + 4]
                    mp = ps.tile([P, len(grp) * P], f32r, tag="ps",
                                 name=f"mp{r}_{b0}")
                    for i, c in enumerate(grp):
                        nc.tensor.transpose(
                            mp[:, i * P:(i + 1) * P],
                            g_sb[:, c, r * P:(r + 1) * P],
                            ident_r[:])
                    evict(ei, g_sb[:, r, grp[0] * P:(grp[-1] + 1) * P],
                          mp[:])
                    ei += 1

            # g = rowsum(G) in two f32r limbs
            with tc.tile_pool(name="pgst", bufs=1) as pgst:
                g_f = pgst.tile([P, DT, 1], f32, tag="gf", name="g_f")
                for dt in range(DT):
                    nc.vector.reduce_sum(g_f[:, dt], g_sb[:, dt],
                                         axis=mybir.AxisListType.X)
                nc.vector.tensor_copy(g_hi[:], g_f[:])
                nc.vector.tensor_sub(g_lo[:], g_f[:], g_hi[:])

            # ------- Phase C: A, t, scores, softmax-from-PSUM -------------
            # ppr: tiles that outlive pc (p_r, wv_r used in E; wv prefetch
            # overlaps C because its buffer can't collide with pc tiles)
            ppr_cm = tc.tile_pool(name="ppr", bufs=1)
            ppr = ppr_cm.__enter__()
            p_r8 = ppr.tile([P, KT, DQ], f8, tag="pr8", name="p_r8")
            wv_r = ppr.tile([P, DT, DKH], f32r, tag="wv", name="wv_r")
            xe0 = ppr.tile([P, DT, ECH], f32, tag="xe0", name="xe0")
            nc.sync.dma_start(xe0[:], xv[:, :, 0:ECH])

            pc_cm = tc.tile_pool(name="pc", bufs=1)
            pc = pc_cm.__enter__()
            wq_r = pc.tile([P, DT, DQ - 256], f32r, tag="wq", name="wq_r")
            wk_r = pc.tile([P, DT, DKH], f32r, tag="wk", name="wk_r")
            a_sb = pc.tile([P, DT, DQ], f32r, tag="a", name="a_sb")
            t_hi = pc.tile([1, DQ], f32r, tag="thi", name="t_hi")
            t_lo = pc.tile([1, DQ], f32r, tag="tlo", name="t_lo")

            with tc.tile_pool(name="pwst", bufs=2) as pwst:
                def wq_ap(dt, q0, q1):
                    # Wq' col range [q0:q1): quarter 0 lives in wq_q0,
                    # the rest in wq_r at offset-256
                    if q1 <= 256:
                        return wq_q0[:, dt, q0:q1]
                    return wq_r[:, dt, q0 - 256:q1 - 256]

                for qq in range(1, DQ // 256):
                    wtmp = pwst.tile([P, DT, 256], f32, tag="wt",
                                     name=f"wt{qq}")
                    (nc.sync, nc.scalar, nc.gpsimd)[qq % 3].dma_start(
                        wtmp[:], wqv[:, :, qq * 256:(qq + 1) * 256])
                    nc.vector.tensor_copy(
                        wq_r[:, :, (qq - 1) * 256:qq * 256], wtmp[:])
                for dt in range(0, DT, 2):
                    d2 = bass.ds(dt, 2)
                    wtmp2 = pwst.tile([P, 2, DKH], f32, tag="wt2",
                                      name=f"wt2{dt}")
                    (nc.scalar, nc.gpsimd)[(dt // 2) % 2].dma_start(
                        wtmp2[:], wkv[:, d2])
                    nc.gpsimd.tensor_copy(wk_r[:, d2], wtmp2[:])
                    wtmp3 = pwst.tile([P, 2, DKH], f32, tag="wt3",
                                      name=f"wt3{dt}")
                    (nc.gpsimd, nc.sync)[(dt // 2) % 2].dma_start(
                        wtmp3[:], wvv[:, d2])
                    nc.gpsimd.tensor_copy(wv_r[:, d2], wtmp3[:])

                # A^T[d', q] = sum_d G[d, d'] Wq'^T[d, q], in 256-wide
                # quarters: quarter 0 uses the prefetched wq_q0 and starts
                # right at G-end, overlapping the wq_r load
                for qq in range(DQ // 256):
                    for dpt in reversed(range(DT)):
                        dsl = bass.ds(dpt * P, P)
                        ap_ = ps.tile([P, 256], f32, tag="ps",
                                      name=f"ap{dpt}_{qq}")
                        for dt in range(DT):
                            nc.tensor.matmul(
                                ap_[:], g_sb[:, dt, dsl],
                                wq_ap(dt, qq * 256, (qq + 1) * 256),
                                start=(dt == 0), stop=(dt == DT - 1))
                        evict(ei, a_sb[:, dpt, bass.ds(qq * 256, 256)],
                              ap_[:])
                        ei += 1

                # t[q] = 0.5 * (g_hi + g_lo)^T Wq'  (two f32r limbs)
                for qq in range(DQ // 256):
                    qsl = bass.ds(qq * 256, 256)
                    tp2 = ps.tile([P, 256], f32, tag="ps", name=f"tq{qq}")
                    first = True
                    for limb in (g_hi, g_lo):
                        for dt in range(DT):
                            nc.tensor.matmul(
                                tp2[0:1, :], limb[:, dt],
                                wq_ap(dt, qq * 256, (qq + 1) * 256),
                                start=first,
                                stop=(limb is g_lo and dt == DT - 1))
                            first = False
                    nc.scalar.mul(t_hi[:, qsl], tp2[0:1, :], 0.5)
                    nc.vector.scalar_tensor_tensor(
                        t_lo[:, qsl], tp2[0:1, :], 0.5, t_hi[:, qsl],
                        op0=mybir.AluOpType.mult,
                        op1=mybir.AluOpType.subtract)

            with (
                tc.tile_pool(name="psmx", bufs=2) as psmx,
                tc.tile_pool(name="pstat", bufs=4) as pstat,
            ):
                # scoresT[k, q] = Wk'^T A^T + ones x (t_hi + t_lo);
                # softmax over q straight from the two q-chunk PSUMs
                for kt in range(KT):
                    ksl = bass.ds(kt * P, P)
                    sp = []
                    for qc in range(QC):
                        qsl = bass.ds(qc * 512, 512)
                        s = psg.tile([P, 512], f32, tag="psg",
                                     name=f"sp{kt}_{qc}")
                        for dpt in range(DT):
                            nc.tensor.matmul(
                                s[:], wk_r[:, dpt, ksl], a_sb[:, dpt, qsl],
                                start=(dpt == 0), stop=False)
                        nc.tensor.matmul(s[:], ones_row[:], t_hi[:, qsl],
                                         start=False, stop=False)
                        nc.tensor.matmul(s[:], ones_row[:], t_lo[:, qsl],
                                         start=False, stop=True)
                        sp.append(s)
                    m0 = pstat.tile([P, 1], f32, tag="m0")
                    m1 = pstat.tile([P, 1], f32, tag="m1")
                    negm = pstat.tile([P, 1], f32, tag="negm")
                    den0 = pstat.tile([P, 1], f32, tag="den0")
                    den1 = pstat.tile([P, 1], f32, tag="den1")
                    rden = pstat.tile([P, 1], f32, tag="rden")
                    nc.vector.reduce_max(m0[:], sp[0][:],
                                         axis=mybir.AxisListType.X)
                    nc.vector.reduce_max(m1[:], sp[1][:],
                                         axis=mybir.AxisListType.X)
                    nc.vector.tensor_max(m0[:], m0[:], m1[:])
                    nc.vector.tensor_scalar_mul(negm[:], m0[:], -scale)
                    e0 = psmx.tile([P, 512], f32, tag="e0")
                    e1 = psmx.tile([P, 512], f32, tag="e1")
                    nc.scalar.activation(
                        e0[:], sp[0][:], mybir.ActivationFunctionType.Exp,
                        bias=negm[:], scale=scale, accum_out=den0[:])
                    nc.scalar.activation(
                        e1[:], sp[1][:], mybir.ActivationFunctionType.Exp,
                        bias=negm[:], scale=scale, accum_out=den1[:])
                    nc.vector.tensor_add(den0[:], den0[:], den1[:])
                    nc.vector.reciprocal(rden[:], den0[:])
                    nc.vector.tensor_scalar_mul(p_r8[:, kt, 0:512], e0[:],
                                                rden[:])
                    nc.vector.tensor_scalar_mul(p_r8[:, kt, 512:DQ], e1[:],
                                                rden[:])
            pc_cm.__exit__(None, None, None)

            # ------- Phase E: V proj fused with out -----------------------
            with (
                tc.tile_pool(name="pex", bufs=2) as pex,
                tc.tile_pool(name="pev", bufs=2) as pev,
                tc.tile_pool(name="pout", bufs=4) as pout,
                tc.tile_pool(name="pseed", bufs=1) as pseed,
            ):
                seed_sb = pseed.tile([1, 1], f32, tag="seed")
                nc.sync.dma_start(seed_sb[:], seed.ap())
                outv = out.ap().rearrange("(qt p) n -> p qt n", p=P)

                for c in range(NCE):
                    ncol = bass.ds(c * ECH, ECH)
                    if c == 0:
                        xc2 = xe0
                    else:
                        xc2 = pex.tile([P, DT, ECH], f32, tag="xc2",
                                       name=f"xe{c}")
                        nc.sync.dma_start(xc2[:], xv[:, :, ncol])
                    xr2 = pex.tile([P, DT, ECH], f32r, tag="xr2",
                                   name=f"xre{c}")
                    nc.scalar.copy(xr2[:], xc2[:])

                    v_sb = pev.tile([P, KT, ECH], f8, tag="v", name=f"v{c}")
                    for vt in range(KT):
                        vp = ps.tile([P, ECH], f32, tag="ps",
                                     name=f"vp{c}_{vt}")
                        vsl = bass.ds(vt * P, P)
                        for dt in range(DT):
                            nc.tensor.matmul(
                                vp[:], wv_r[:, dt, vsl], xr2[:, dt],
                                start=(dt == 0), stop=(dt == DT - 1))
                        evict(ei, v_sb[:, vt], vp[:])
                        ei += 1

                    for qg in range(QT128 // 4):
                        osb = pout.tile([P, 4, ECH], bf16, tag="osb")
                        for qi in range(4):
                            qt = qg * 4 + qi
                            op = psg.tile([P, ECH], f32, tag="psg",
                                          name=f"op{c}_{qt}")
                            qsl2 = bass.ds(qt * P, P)
                            for kp in range(KT // 2):
                                nc.tensor.matmul(
                                    op[:],
                                    p_r8[:, 2 * kp:2 * kp + 2, qsl2],
                                    v_sb[:, 2 * kp:2 * kp + 2, :],
                                    start=(kp == 0), stop=(kp == KT // 2 - 1),
                                    perf_mode=mybir.MatmulPerfMode.DoubleRow)
                            nc.vector.tensor_copy(osb[:, qi], op[:])
                            if c == 0 and qt == 0:
                                nc.vector.tensor_scalar_add(
                                    osb[0:1, 0, 0:1], op[0:1, 0:1],
                                    seed_sb[:])
                        nc.sync.dma_start(
                            outv[:, qg * 4:(qg + 1) * 4, ncol], osb[:])

            ppr_cm.__exit__(None, None, None)
            pwq0_cm.__exit__(None, None, None)
            pg_cm.__exit__(None, None, None)
            if rep_cm is not None:
                rep_cm.__exit__(None, None, None)
            if sink is not None:
                # touch every out-DMA region (walrus DCE is region-precise):
                # one full row per q-half covers all (qg, c) blocks
                with tc.tile_pool(name="psink", bufs=1) as psink:
                    row0 = psink.tile([1, N], bf16, tag="r0", name="row0")
                    row1 = psink.tile([1, N], bf16, tag="r1", name="row1")
                    nc.sync.dma_start(row0[:], out.ap()[0:1, :])
                    nc.sync.dma_start(row1[:], out.ap()[DQ // 2:DQ // 2 + 1, :])
                    s0 = psink.tile([1, 1], f32, tag="s0", name="s0")
                    s1 = psink.tile([1, 1], f32, tag="s1", name="s1")
                    nc.vector.reduce_sum(s0[:], row0[:],
                                         axis=mybir.AxisListType.X)
                    nc.vector.reduce_sum(s1[:], row1[:],
                                         axis=mybir.AxisListType.X)
                    nc.vector.tensor_add(s0[:], s0[:], s1[:])
                    nc.sync.dma_start(sink.ap(), s0[:])
            p0_cm.__exit__(None, None, None)

    nc.compile()
    return nc


_CACHE = {}


def _get_nc(DX, N, DQ, DKH):
    key = (DX, N, DQ, DKH)
    if key not in _CACHE:
        _CACHE[key] = _build_core_kernel(DX, N, DQ, DKH)
    return _CACHE[key]


def _run(x, Wq, Wk, Wv, **spmd_kwargs):
    from concourse.bass_utils import run_bass_kernel_spmd

    B, DX, N = x.shape
    DQ = Wq.shape[0]
    DK = Wk.shape[0]
    assert (B, DX, N, DQ, DK) == (B_FULL, DX_FULL, N_FULL, DQ_FULL, DK_FULL)
    DKH = DK // 2

    nc = _get_nc(DX, N, DQ, DKH)

    # Wq/Wk shipped mean-removed (entries - 0.5); the q-varying part of the
    # mean term is restored on-chip via t[q] (see module docstring)
    WqT = np.ascontiguousarray(Wq.T, dtype=np.float32) - np.float32(0.5)
    WkT = np.ascontiguousarray(Wk.T, dtype=np.float32) - np.float32(0.5)
    WvT = np.ascontiguousarray(Wv.T, dtype=np.float32)
    eye = np.eye(128, dtype=np.float32)

    in_maps = []
    for c in range(N_CORES):
        b, h = divmod(c, 2)
        hsl = slice(h * DKH, (h + 1) * DKH)
        in_maps.append({
            "xb": np.ascontiguousarray(x[b], dtype=np.float32),
            "wqt": WqT,
            "wkt": np.ascontiguousarray(WkT[:, hsl]),
            "wvt": np.ascontiguousarray(WvT[:, hsl]),
            "ident": eye,
            "seed": np.zeros((1, 1), np.float32),
        })

    res = run_bass_kernel_spmd(nc, in_maps, core_ids=list(range(N_CORES)),
                               **spmd_kwargs)
    out = np.empty((B, DQ, N), np.float32)
    for b in range(B):
        out[b] = (np.asarray(res.results[2 * b]["out"]).astype(np.float32)
                  + np.asarray(res.results[2 * b + 1]["out"]).astype(
                      np.float32))
    return out, res


def kernel(x, Wq, Wk, Wv):
    return _run(x, Wq, Wk, Wv)[0]


# revision 8
# speedup vs baseline: 1.4748x; 1.1137x over previous
"""TRN2 Bass kernel for nn_Attention_369367187796 — Gram-route scores.

Reference (B=4, DX=1024, N=4096, DQ=DK=DV=1024, fp32):
    Q = Wq @ x[b]; K = Wk @ x[b]; V = Wv @ x[b]
    scores = Q @ K.T   (contract n)
    p = softmax(scores / sqrt(DQ), axis=q)   <- softmax over q
    out[q,n] = sum_k p[q,k] V[k,n]

Key algebra: Q and K are used ONLY in scores, and
    scores = Wq (x x^T) Wk^T
so per batch the Q/K/scores path costs 4.3+1.1+1.1 GMAC via the Gram matrix
G = x x^T instead of 12.9 GMAC for Qproj+Kproj+scores. With softmax over q,
any score term constant across q drops out, so mean-removed weights
(Wq' = Wq-0.5, Wk' = Wk-0.5) need only the rank-1 q-varying correction
    t[q] = 0.5 * sum_d Wq'[q,d] g[d],   g = rowsum(G)
restored (two f32r limbs; K-side and const terms cancel in softmax).

Precision (numpy-simulated; sim matches HW for the old direct scheme to
1e-5): single-limb f32r everywhere (G, A=Wq'G, Wk', V, p) gives end-to-end
rel err ~8e-4 vs fp64 (HW-verified) — 25x under the 2e-2 gate.

Sharding: 8 cores = 4 batches x 2 k-halves (DKH=512). Each core computes the
full G/A (duplicated within the pair), its k-half of scores/softmax/V, and
partial out[q,n] summed on the host — no cross-core communication.

Per-core phases:
  A: stream x (256-col chunks), PE-transpose raw f32 -> xT (f32r on evict);
     the first G generation (rows 0-3 x cols 0:512) accumulates in a
     dedicated 4-bank PSUM pool as xT tiles land, keeping PE dense
  B: remaining G generations (32-matmul PSUM chains), mirror lower-left via
     PE-transpose (G symmetric), g = rowsum(G)
  C: A^T[d',q] = G Wq'^T; scoresT[k,q] = Wk'^T_half A^T + ones x (th+tl);
     softmax over q straight from the score PSUMs
  E: stream x again: V k-half projection fused with out = p^T V, DMA out
"""

import math

import numpy as np

B_FULL, DX_FULL, N_FULL = 4, 1024, 4096
DQ_FULL = DK_FULL = 1024
N_CORES = 8


def _build_core_kernel(DX, N, DQ, DKH, bench=False, bench_reps=0):
    import concourse.bass as bass
    import concourse.mybir as mybir
    import concourse.tile as tile
    from concourse import bacc

    f32 = mybir.dt.float32
    f32r = mybir.dt.float32r
    f8 = mybir.dt.float8e4
    bf16 = mybir.dt.bfloat16

    P = 128
    DT = DX // P            # 8 d-tiles (also d' tiles)
    NT = N // P             # 32 n-tiles
    CHA = 256               # phase A chunk cols
    NCA = N // CHA          # 16 chunks
    ECH = 256               # phase E chunk cols
    NCE = N // ECH          # 16 chunks
    KT = DKH // P           # 4 k-tiles
    QT128 = DQ // P         # 8 q-tiles
    QC = DQ // 512          # 2 q-chunks
    scale = 1.0 / math.sqrt(DQ)

    assert DX % P == 0 and N % ECH == 0 and DQ % 512 == 0 and DKH % P == 0

    nc = bacc.Bacc(None, target_bir_lowering=False, debug=False)

    kind_big = "Internal" if bench else "ExternalInput"
    # In bench mode out is Internal (only [1,1] seed/sink cross the tunnel);
    # the post-loop readback of out[0,0] into sink keeps every out write
    # live — without it the compiler dead-code-eliminates most of phase E
    # (observed as a physically impossible 174us/iter).
    kind_out = "Internal" if bench else "ExternalOutput"
    xb = nc.dram_tensor("xb", [DX, N], f32, kind=kind_big)
    wqt = nc.dram_tensor("wqt", [DX, DQ], f32, kind=kind_big)
    wkt = nc.dram_tensor("wkt", [DX, DKH], f32, kind=kind_big)
    wvt = nc.dram_tensor("wvt", [DX, DKH], f32, kind=kind_big)
    # identity for PE transposes: tiny, stays ExternalInput in bench mode too
    ident = nc.dram_tensor("ident", [P, P], f32, kind="ExternalInput")
    seed = nc.dram_tensor("seed", [1, 1], f32, kind="ExternalInput")
    # out in bf16: halves the 16MB output write (phase E is DMA-bound
    # after the fp8 out-matmul); host upcasts to f32. Adds ~2e-3 rel err.
    out = nc.dram_tensor("out", [DQ, N], bf16, kind=kind_out)
    sink = (nc.dram_tensor("sink", [1, 1], f32, kind="ExternalOutput")
            if bench else None)

    xv = xb.ap().rearrange("(dt p) n -> p dt n", p=P)
    wqv = wqt.ap().rearrange("(dt p) q -> p dt q", p=P)
    wkv = wkt.ap().rearrange("(dt p) k -> p dt k", p=P)
    wvv = wvt.ap().rearrange("(dt p) k -> p dt k", p=P)

    with tile.TileContext(nc) as tc:
        with (
            tc.tile_pool(name="ps", bufs=4, space="PSUM") as ps,
            tc.tile_pool(name="psg", bufs=4, space="PSUM") as psg,
        ):
            p0_cm = tc.tile_pool(name="pres0", bufs=1)
            p0 = p0_cm.__enter__()
            ident_r = p0.tile([P, P], f32r, tag="idr", name="ident_r")
            ident_f = p0.tile([P, P], f32, tag="idf", name="ident_f")
            ones_row = p0.tile([1, P], f32r, tag="ones", name="ones_row")
            g_hi = p0.tile([P, DT, 1], f32r, tag="ghi", name="g_hi")
            g_lo = p0.tile([P, DT, 1], f32r, tag="glo", name="g_lo")

            rep_cm = tc.For_i(0, bench_reps, 1) if bench_reps else None
            if rep_cm is not None:
                rep_cm.__enter__()

            # engine rotation for PSUM->SBUF evictions (GPSIMD can't
            # read PSUM, so alternate DVE and Act)
            def evict(i, dst, src):
                if i % 2 == 0:
                    nc.vector.tensor_copy(dst, src)
                else:
                    nc.scalar.copy(dst, src)

            # prologue: identity + ones (f32r via rounding compute)
            with tc.tile_pool(name="ppro", bufs=1) as ppro:
                istage = ppro.tile([P, P], f32, tag="ist", name="istage")
                nc.sync.dma_start(istage[:], ident.ap())
                nc.sync.dma_start(ident_f[:], ident.ap())
                nc.vector.tensor_copy(ident_r[:], istage[:])
                nc.gpsimd.memset(istage[:, 0:P], 1.0)
                nc.vector.tensor_copy(ones_row[:], istage[0:1, 0:P])

            # ------- Phase A: xT = round(x)^T; G gen0 chases the chunks ----
            pg_cm = tc.tile_pool(name="pg", bufs=1)
            pg = pg_cm.__enter__()
            g_sb = pg.tile([P, DT, DX], f32r, tag="g", name="g_sb")

            # first Wq' quarter prefetched at body start so A-matmuls can
            # begin the moment G completes (the rest of Wq' loads into pc,
            # whose SBUF region only frees when xT dies)
            pwq0_cm = tc.tile_pool(name="pwq0", bufs=1)
            pwq0 = pwq0_cm.__enter__()
            wq_q0 = pwq0.tile([P, DT, 256], f32r, tag="wq0", name="wq_q0")

            pxt_cm = tc.tile_pool(name="pxt", bufs=1)
            pxt = pxt_cm.__enter__()
            xt = pxt.tile([P, NT, DX], f32r, tag="xt", name="xt")

            # gen0: G rows 0-1 x all cols, accumulated as chunks land
            GEN0 = [(0, 0), (0, 1), (1, 0), (1, 1)]   # (row, col-half)
            gp0 = [psg.tile([P, 512], f32, tag="psg", name=f"gp0_{m}")
                   for m in range(4)]

            ei = 0
            with (
                tc.tile_pool(name="pxa", bufs=3) as pxa,
                tc.tile_pool(name="pwq0st", bufs=1) as pwq0st,
            ):
                for c in range(NCA):
                    ncol = bass.ds(c * CHA, CHA)
                    xc = pxa.tile([P, DT, CHA], f32, tag="xc", name=f"xc{c}")
                    if c == 0:
                        nc.sync.dma_start(xc[:, :, 0:CHA // 2],
                                          xv[:, :, 0:CHA // 2])
                        nc.sync.dma_start(xc[:, :, CHA // 2:CHA],
                                          xv[:, :, CHA // 2:CHA])
                    else:
                        nc.sync.dma_start(xc[:], xv[:, :, ncol])
                    if c == 1:
                        w0tmp = pwq0st.tile([P, DT, 256], f32, tag="w0t",
                                            name="w0t")
                        nc.scalar.dma_start(w0tmp[:], wqv[:, :, 0:256])
                        nc.gpsimd.tensor_copy(wq_q0[:], w0tmp[:])
                    for j in range(CHA // P):
                        nt = c * (CHA // P) + j
                        for dh in range(DT // 4):
                            tp = ps.tile([P, 512], f32, tag="ps",
                                         name=f"tp{nt}_{dh}")
                            for di in range(4):
                                dt = dh * 4 + di
                                nc.tensor.transpose(
                                    tp[:, di * P:(di + 1) * P],
                                    xc[:, dt, j * P:(j + 1) * P],
                                    ident_f[:])
                            evict(ei, xt[:, nt, dh * 512:(dh + 1) * 512],
                                  tp[:])
                            ei += 1
                        for m, (row, ch) in enumerate(GEN0):
                            nc.tensor.matmul(
                                gp0[m][:], xt[:, nt, row * P:(row + 1) * P],
                                xt[:, nt, ch * 512:(ch + 1) * 512],
                                start=(nt == 0), stop=(nt == NT - 1))

            # ------- Phase B: remaining G generations, mirror, g ----------
            for m, (row, ch) in enumerate(GEN0):
                evict(ei, g_sb[:, row, ch * 512:(ch + 1) * 512], gp0[m][:])
                ei += 1
            # upper-triangle ragged blocks (row, c0, c1); lower-left comes
            # from the mirror (G symmetric)
            GENS = [(2, 256, 768), (3, 256, 768), (4, 512, 1024),
                    (5, 512, 1024), (2, 768, 1024), (3, 768, 1024),
                    (6, 768, 1024), (7, 768, 1024)]
            for row, c0, c1 in GENS:
                gp = ps.tile([P, c1 - c0], f32, tag="ps",
                             name=f"gp{row}_{c0}")
                for nt in range(NT):
                    nc.tensor.matmul(
                        gp[:], xt[:, nt, row * P:(row + 1) * P],
                        xt[:, nt, bass.ds(c0, c1 - c0)],
                        start=(nt == 0), stop=(nt == NT - 1))
                evict(ei, g_sb[:, row, c0:c1], gp[:])
                ei += 1
            pxt_cm.__exit__(None, None, None)  # free xT (16MB)

            # mirror: G[r, c*128:] = G[c, r*128:]^T for tiles left of each
            # row's directly-computed range
            ROW_START = [0, 0, 2, 2, 4, 4, 6, 6]   # first computed col-tile
            for r in range(2, DT):
                cs = list(range(ROW_START[r]))
                for b0 in range(0, len(cs), 4):
                    grp = cs[b0:b0 + 4]
                    mp = ps.tile([P, len(grp) * P], f32r, tag="ps",
                                 name=f"mp{r}_{b0}")
                    for i, c in enumerate(grp):
                        nc.tensor.transpose(
                            mp[:, i * P:(i + 1) * P],
                            g_sb[:, c, r * P:(r + 1) * P],
                            ident_r[:])
                    evict(ei, g_sb[:, r, grp[0] * P:(grp[-1] + 1) * P],
                          mp[:])
                    ei += 1

            # g = rowsum(G) in two f32r limbs
            with tc.tile_pool(name="pgst", bufs=1) as pgst:
                g_f = pgst.tile([P, DT, 1], f32, tag="gf", name="g_f")
                for dt in range(DT):
                    nc.vector.reduce_sum(g_f[:, dt], g_sb[:, dt],
                                         axis=mybir.AxisListType.X)
                nc.vector.tensor_copy(g_hi[:], g_f[:])
                nc.vector.tensor_sub(g_lo[:], g_f[:], g_hi[:])

            # ------- Phase C: A, t, scores, softmax-from-PSUM -------------
            # ppr: tiles that outlive pc (p_r, wv_r used in E; wv prefetch
            # overlaps C because its buffer can't collide with pc tiles)
            ppr_cm = tc.tile_pool(name="ppr", bufs=1)
            ppr = ppr_cm.__enter__()
            p_r8 = ppr.tile([P, KT, DQ], f8, tag="pr8", name="p_r8")
            wv_r = ppr.tile([P, DT, DKH], f32r, tag="wv", name="wv_r")
            xe0 = ppr.tile([P, DT, ECH], f32, tag="xe0", name="xe0")
            nc.sync.dma_start(xe0[:], xv[:, :, 0:ECH])

            pc_cm = tc.tile_pool(name="pc", bufs=1)
            pc = pc_cm.__enter__()
            wq_r = pc.tile([P, DT, DQ - 256], f32r, tag="wq", name="wq_r")
            wk_r = pc.tile([P, DT, DKH], f32r, tag="wk", name="wk_r")
            a_sb = pc.tile([P, DT, DQ], f32r, tag="a", name="a_sb")
            t_hi = pc.tile([1, DQ], f32r, tag="thi", name="t_hi")
            t_lo = pc.tile([1, DQ], f32r, tag="tlo", name="t_lo")

            with tc.tile_pool(name="pwst", bufs=2) as pwst:
                def wq_ap(dt, q0, q1):
                    # Wq' col range [q0:q1): quarter 0 lives in wq_q0,
                    # the rest in wq_r at offset-256
                    if q1 <= 256:
                        return wq_q0[:, dt, q0:q1]
                    return wq_r[:, dt, q0 - 256:q1 - 256]

                for qq in range(1, DQ // 256):
                    wtmp = pwst.tile([P, DT, 256], f32, tag="wt",
                                     name=f"wt{qq}")
                    (nc.sync, nc.scalar, nc.gpsimd)[qq % 3].dma_start(
                        wtmp[:], wqv[:, :, qq * 256:(qq + 1) * 256])
                    nc.vector.tensor_copy(
                        wq_r[:, :, (qq - 1) * 256:qq * 256], wtmp[:])
                for dt in range(0, DT, 2):
                    d2 = bass.ds(dt, 2)
                    wtmp2 = pwst.tile([P, 2, DKH], f32, tag="wt2",
                                      name=f"wt2{dt}")
                    (nc.scalar, nc.gpsimd)[(dt // 2) % 2].dma_start(
                        wtmp2[:], wkv[:, d2])
                    nc.gpsimd.tensor_copy(wk_r[:, d2], wtmp2[:])
                    wtmp3 = pwst.tile([P, 2, DKH], f32, tag="wt3",
                                      name=f"wt3{dt}")
                    (nc.gpsimd, nc.sync)[(dt // 2) % 2].dma_start(
                        wtmp3[:], wvv[:, d2])
                    nc.gpsimd.tensor_copy(wv_r[:, d2], wtmp3[:])

                # A^T[d', q] = sum_d G[d, d'] Wq'^T[d, q], in 256-wide
                # quarters: quarter 0 uses the prefetched wq_q0 and starts
                # right at G-end, overlapping the wq_r load
                for qq in range(DQ // 256):
                    for dpt in reversed(range(DT)):
                        dsl = bass.ds(dpt * P, P)
                        ap_ = ps.tile([P, 256], f32, tag="ps",
                                      name=f"ap{dpt}_{qq}")
                        for dt in range(DT):
                            nc.tensor.matmul(
                                ap_[:], g_sb[:, dt, dsl],
                                wq_ap(dt, qq * 256, (qq + 1) * 256),
                                start=(dt == 0), stop=(dt == DT - 1))
                        evict(ei, a_sb[:, dpt, bass.ds(qq * 256, 256)],
                              ap_[:])
                        ei += 1

                # t[q] = 0.5 * (g_hi + g_lo)^T Wq'  (two f32r limbs)
                for qq in range(DQ // 256):
                    qsl = bass.ds(qq * 256, 256)
                    tp2 = ps.tile([P, 256], f32, tag="ps", name=f"tq{qq}")
                    first = True
                    for limb in (g_hi, g_lo):
                        for dt in range(DT):
                            nc.tensor.matmul(
                                tp2[0:1, :], limb[:, dt],
                                wq_ap(dt, qq * 256, (qq + 1) * 256),
                                start=first,
                                stop=(limb is g_lo and dt == DT - 1))
                            first = False
                    nc.scalar.mul(t_hi[:, qsl], tp2[0:1, :], 0.5)
                    nc.vector.scalar_tensor_tensor(
                        t_lo[:, qsl], tp2[0:1, :], 0.5, t_hi[:, qsl],
                        op0=mybir.AluOpType.mult,
                        op1=mybir.AluOpType.subtract)

            with (
                tc.tile_pool(name="psmx", bufs=2) as psmx,
                tc.tile_pool(name="pstat", bufs=4) as pstat,
            ):
                # scoresT[k, q] = Wk'^T A^T + ones x (t_hi + t_lo);
                # softmax over q straight from the two q-chunk PSUMs
                for kt in range(KT):
                    ksl = bass.ds(kt * P, P)
                    sp = []
                    for qc in range(QC):
                        qsl = bass.ds(qc * 512, 512)
                        s = psg.tile([P, 512], f32, tag="psg",
                                     name=f"sp{kt}_{qc}")
                        for dpt in range(DT):
                            nc.tensor.matmul(
                                s[:], wk_r[:, dpt, ksl], a_sb[:, dpt, qsl],
                                start=(dpt == 0), stop=False)
                        nc.tensor.matmul(s[:], ones_row[:], t_hi[:, qsl],
                                         start=False, stop=False)
                        nc.tensor.matmul(s[:], ones_row[:], t_lo[:, qsl],
                                         start=False, stop=True)
                        sp.append(s)
                    m0 = pstat.tile([P, 1], f32, tag="m0")
                    m1 = pstat.tile([P, 1], f32, tag="m1")
                    negm = pstat.tile([P, 1], f32, tag="negm")
                    den0 = pstat.tile([P, 1], f32, tag="den0")
                    den1 = pstat.tile([P, 1], f32, tag="den1")
                    rden = pstat.tile([P, 1], f32, tag="rden")
                    nc.vector.reduce_max(m0[:], sp[0][:],
                                         axis=mybir.AxisListType.X)
                    nc.vector.reduce_max(m1[:], sp[1][:],
                                         axis=mybir.AxisListType.X)
                    nc.vector.tensor_max(m0[:], m0[:], m1[:])
                    nc.vector.tensor_scalar_mul(negm[:], m0[:], -scale)
                    e0 = psmx.tile([P, 512], f32, tag="e0")
                    e1 = psmx.tile([P, 512], f32, tag="e1")
                    nc.scalar.activation(
                        e0[:], sp[0][:], mybir.ActivationFunctionType.Exp,
                        bias=negm[:], scale=scale, accum_out=den0[:])
                    nc.scalar.activation(
                        e1[:], sp[1][:], mybir.ActivationFunctionType.Exp,
                        bias=negm[:], scale=scale, accum_out=den1[:])
                    nc.vector.tensor_add(den0[:], den0[:], den1[:])
                    nc.vector.reciprocal(rden[:], den0[:])
                    nc.vector.tensor_scalar_mul(p_r8[:, kt, 0:512], e0[:],
                                                rden[:])
                    nc.vector.tensor_scalar_mul(p_r8[:, kt, 512:DQ], e1[:],
                                                rden[:])
            pc_cm.__exit__(None, None, None)

            # ------- Phase E: V proj fused with out -----------------------
            with (
                tc.tile_pool(name="pex", bufs=3) as pex,
                tc.tile_pool(name="pev", bufs=3) as pev,
                tc.tile_pool(name="pout", bufs=4) as pout,
                tc.tile_pool(name="pseed", bufs=1) as pseed,
            ):
                seed_sb = pseed.tile([1, 1], f32, tag="seed")
                nc.sync.dma_start(seed_sb[:], seed.ap())
                outv = out.ap().rearrange("(qt p) n -> p qt n", p=P)

                for c in range(NCE):
                    ncol = bass.ds(c * ECH, ECH)
                    if c == 0:
                        xc2 = xe0
                    else:
                        xc2 = pex.tile([P, DT, ECH], f32, tag="xc2",
                                       name=f"xe{c}")
                        nc.sync.dma_start(xc2[:], xv[:, :, ncol])
                    xr2 = pex.tile([P, DT, ECH], f32r, tag="xr2",
                                   name=f"xre{c}")
                    nc.scalar.copy(xr2[:], xc2[:])

                    v_sb = pev.tile([P, KT, ECH], f8, tag="v", name=f"v{c}")
                    for vt in range(KT):
                        vp = ps.tile([P, ECH], f32, tag="ps",
                                     name=f"vp{c}_{vt}")
                        vsl = bass.ds(vt * P, P)
                        for dt in range(DT):
                            nc.tensor.matmul(
                                vp[:], wv_r[:, dt, vsl], xr2[:, dt],
                                start=(dt == 0), stop=(dt == DT - 1))
                        evict(ei, v_sb[:, vt], vp[:])
                        ei += 1

                    for qg in range(QT128 // 4):
                        osb = pout.tile([P, 4, ECH], bf16, tag="osb")
                        for qi in range(4):
                            qt = qg * 4 + qi
                            op = psg.tile([P, ECH], f32, tag="psg",
                                          name=f"op{c}_{qt}")
                            qsl2 = bass.ds(qt * P, P)
                            for kp in range(KT // 2):
                                nc.tensor.matmul(
                                    op[:],
                                    p_r8[:, 2 * kp:2 * kp + 2, qsl2],
                                    v_sb[:, 2 * kp:2 * kp + 2, :],
                                    start=(kp == 0), stop=(kp == KT // 2 - 1),
                                    perf_mode=mybir.MatmulPerfMode.DoubleRow)
                            nc.vector.tensor_copy(osb[:, qi], op[:])
                            if c == 0 and qt == 0:
                                nc.vector.tensor_scalar_add(
                                    osb[0:1, 0, 0:1], op[0:1, 0:1],
                                    seed_sb[:])
                        nc.sync.dma_start(
                            outv[:, qg * 4:(qg + 1) * 4, ncol], osb[:])

            ppr_cm.__exit__(None, None, None)
            pwq0_cm.__exit__(None, None, None)
            pg_cm.__exit__(None, None, None)
            if rep_cm is not None:
                rep_cm.__exit__(None, None, None)
            if sink is not None:
                # touch every out-DMA region (walrus DCE is region-precise):
                # one full row per q-half covers all (qg, c) blocks
                with tc.tile_pool(name="psink", bufs=1) as psink:
                    row0 = psink.tile([1, N], bf16, tag="r0", name="row0")
                    row1 = psink.tile([1, N], bf16, tag="r1", name="row1")
                    nc.sync.dma_start(row0[:], out.ap()[0:1, :])
                    nc.sync.dma_start(row1[:], out.ap()[DQ // 2:DQ // 2 + 1, :])
                    s0 = psink.tile([1, 1], f32, tag="s0", name="s0")
                    s1 = psink.tile([1, 1], f32, tag="s1", name="s1")
                    nc.vector.reduce_sum(s0[:], row0[:],
                                         axis=mybir.AxisListType.X)
                    nc.vector.reduce_sum(s1[:], row1[:],
                                         axis=mybir.AxisListType.X)
                    nc.vector.tensor_add(s0[:], s0[:], s1[:])
                    nc.sync.dma_start(sink.ap(), s0[:])
            p0_cm.__exit__(None, None, None)

    nc.compile()
    return nc


_CACHE = {}


def _get_nc(DX, N, DQ, DKH):
    key = (DX, N, DQ, DKH)
    if key not in _CACHE:
        _CACHE[key] = _build_core_kernel(DX, N, DQ, DKH)
    return _CACHE[key]


def _run(x, Wq, Wk, Wv, **spmd_kwargs):
    from concourse.bass_utils import run_bass_kernel_spmd

    B, DX, N = x.shape
    DQ = Wq.shape[0]
    DK = Wk.shape[0]
    assert (B, DX, N, DQ, DK) == (B_FULL, DX_FULL, N_FULL, DQ_FULL, DK_FULL)
    DKH = DK // 2

    nc = _get_nc(DX, N, DQ, DKH)

    # Wq/Wk shipped mean-removed (entries - 0.5); the q-varying part of the
    # mean term is restored on-chip via t[q] (see module docstring)
    WqT = np.ascontiguousarray(Wq.T, dtype=np.float32) - np.float32(0.5)
    WkT = np.ascontiguousarray(Wk.T, dtype=np.float32) - np.float32(0.5)
    WvT = np.ascontiguousarray(Wv.T, dtype=np.float32)
    eye = np.eye(128, dtype=np.float32)

    in_maps = []
    for c in range(N_CORES):
        b, h = divmod(c, 2)
        hsl = slice(h * DKH, (h + 1) * DKH)
        in_maps.append({
            "xb": np.ascontiguousarray(x[b], dtype=np.float32),
            "wqt": WqT,
            "wkt": np.ascontiguousarray(WkT[:, hsl]),
            "wvt": np.ascontiguousarray(WvT[:, hsl]),
            "ident": eye,
            "seed": np.zeros((1, 1), np.float32),
        })

    res = run_bass_kernel_spmd(nc, in_maps, core_ids=list(range(N_CORES)),
                               **spmd_kwargs)
    out = np.empty((B, DQ, N), np.float32)
    for b in range(B):
        out[b] = (np.asarray(res.results[2 * b]["out"]).astype(np.float32)
                  + np.asarray(res.results[2 * b + 1]["out"]).astype(
                      np.float32))
    return out, res


def kernel(x, Wq, Wk, Wv):
    return _run(x, Wq, Wk, Wv)[0]


# revision 9
# speedup vs baseline: 1.5648x; 1.0610x over previous
"""TRN2 Bass kernel for nn_Attention_369367187796 — Gram-route scores.

Reference (B=4, DX=1024, N=4096, DQ=DK=DV=1024, fp32):
    Q = Wq @ x[b]; K = Wk @ x[b]; V = Wv @ x[b]
    scores = Q @ K.T   (contract n)
    p = softmax(scores / sqrt(DQ), axis=q)   <- softmax over q
    out[q,n] = sum_k p[q,k] V[k,n]

Key algebra: Q and K are used ONLY in scores, and
    scores = Wq (x x^T) Wk^T
so per batch the Q/K/scores path costs 4.3+1.1+1.1 GMAC via the Gram matrix
G = x x^T instead of 12.9 GMAC for Qproj+Kproj+scores. With softmax over q,
any score term constant across q drops out, so mean-removed weights
(Wq' = Wq-0.5, Wk' = Wk-0.5) need only the rank-1 q-varying correction
    t[q] = 0.5 * sum_d Wq'[q,d] g[d],   g = rowsum(G)
restored (two f32r limbs; K-side and const terms cancel in softmax).

Precision (numpy-simulated; sim matches HW for the old direct scheme to
1e-5): single-limb f32r everywhere (G, A=Wq'G, Wk', V, p) gives end-to-end
rel err ~8e-4 vs fp64 (HW-verified) — 25x under the 2e-2 gate.

Sharding: 8 cores = 4 batches x 2 k-halves (DKH=512). Each core computes the
full G/A (duplicated within the pair), its k-half of scores/softmax/V, and
partial out[q,n] summed on the host — no cross-core communication.

Per-core phases:
  A: stream x (256-col chunks), PE-transpose raw f32 -> xT (f32r on evict);
     the first G generation (rows 0-3 x cols 0:512) accumulates in a
     dedicated 4-bank PSUM pool as xT tiles land, keeping PE dense
  B: remaining G generations (32-matmul PSUM chains), mirror lower-left via
     PE-transpose (G symmetric), g = rowsum(G)
  C: A^T[d',q] = G Wq'^T; scoresT[k,q] = Wk'^T_half A^T + ones x (th+tl);
     softmax over q straight from the score PSUMs
  E: stream x again: V k-half projection fused with out = p^T V, DMA out
"""

import math

import numpy as np

B_FULL, DX_FULL, N_FULL = 4, 1024, 4096
DQ_FULL = DK_FULL = 1024
N_CORES = 8


def _build_core_kernel(DX, N, DQ, DKH, bench=False, bench_reps=0):
    import concourse.bass as bass
    import concourse.mybir as mybir
    import concourse.tile as tile
    from concourse import bacc

    f32 = mybir.dt.float32
    f32r = mybir.dt.float32r
    f8 = mybir.dt.float8e4
    bf16 = mybir.dt.bfloat16

    P = 128
    DT = DX // P            # 8 d-tiles (also d' tiles)
    NT = N // P             # 32 n-tiles
    CHA = 256               # phase A chunk cols
    NCA = N // CHA          # 16 chunks
    ECH = 256               # phase E chunk cols
    NCE = N // ECH          # 16 chunks
    KT = DKH // P           # 4 k-tiles
    QT128 = DQ // P         # 8 q-tiles
    QC = DQ // 512          # 2 q-chunks
    scale = 1.0 / math.sqrt(DQ)

    assert DX % P == 0 and N % ECH == 0 and DQ % 512 == 0 and DKH % P == 0

    nc = bacc.Bacc(None, target_bir_lowering=False, debug=False)

    kind_big = "Internal" if bench else "ExternalInput"
    # In bench mode out is Internal (only [1,1] seed/sink cross the tunnel);
    # the post-loop readback of out[0,0] into sink keeps every out write
    # live — without it the compiler dead-code-eliminates most of phase E
    # (observed as a physically impossible 174us/iter).
    kind_out = "Internal" if bench else "ExternalOutput"
    xb = nc.dram_tensor("xb", [DX, N], f32, kind=kind_big)
    wqt = nc.dram_tensor("wqt", [DX, DQ], f32, kind=kind_big)
    wkt = nc.dram_tensor("wkt", [DX, DKH], f32, kind=kind_big)
    wvt = nc.dram_tensor("wvt", [DX, DKH], f32, kind=kind_big)
    # identity for PE transposes: tiny, stays ExternalInput in bench mode too
    ident = nc.dram_tensor("ident", [P, P], f32, kind="ExternalInput")
    seed = nc.dram_tensor("seed", [1, 1], f32, kind="ExternalInput")
    # out in bf16: halves the 16MB output write (phase E is DMA-bound
    # after the fp8 out-matmul); host upcasts to f32. Adds ~2e-3 rel err.
    out = nc.dram_tensor("out", [DQ, N], bf16, kind=kind_out)
    sink = (nc.dram_tensor("sink", [1, 1], f32, kind="ExternalOutput")
            if bench else None)

    xv = xb.ap().rearrange("(dt p) n -> p dt n", p=P)
    wqv = wqt.ap().rearrange("(dt p) q -> p dt q", p=P)
    wkv = wkt.ap().rearrange("(dt p) k -> p dt k", p=P)
    wvv = wvt.ap().rearrange("(dt p) k -> p dt k", p=P)

    with tile.TileContext(nc) as tc:
        with (
            tc.tile_pool(name="ps", bufs=4, space="PSUM") as ps,
            tc.tile_pool(name="psg", bufs=4, space="PSUM") as psg,
        ):
            p0_cm = tc.tile_pool(name="pres0", bufs=1)
            p0 = p0_cm.__enter__()
            ident_r = p0.tile([P, P], f32r, tag="idr", name="ident_r")
            ident_f = p0.tile([P, P], f32, tag="idf", name="ident_f")
            ones_row = p0.tile([1, P], f32r, tag="ones", name="ones_row")
            g_hi = p0.tile([P, DT, 1], f32r, tag="ghi", name="g_hi")
            g_lo = p0.tile([P, DT, 1], f32r, tag="glo", name="g_lo")

            rep_cm = tc.For_i(0, bench_reps, 1) if bench_reps else None
            if rep_cm is not None:
                rep_cm.__enter__()

            # engine rotation for PSUM->SBUF evictions (GPSIMD can't
            # read PSUM, so alternate DVE and Act)
            def evict(i, dst, src):
                if i % 2 == 0:
                    nc.vector.tensor_copy(dst, src)
                else:
                    nc.scalar.copy(dst, src)

            # prologue: identity + ones (f32r via rounding compute)
            with tc.tile_pool(name="ppro", bufs=1) as ppro:
                istage = ppro.tile([P, P], f32, tag="ist", name="istage")
                nc.sync.dma_start(istage[:], ident.ap())
                nc.sync.dma_start(ident_f[:], ident.ap())
                nc.vector.tensor_copy(ident_r[:], istage[:])
                nc.gpsimd.memset(istage[:, 0:P], 1.0)
                nc.vector.tensor_copy(ones_row[:], istage[0:1, 0:P])

            # ------- Phase A: xT = round(x)^T; G gen0 chases the chunks ----
            pg_cm = tc.tile_pool(name="pg", bufs=1)
            pg = pg_cm.__enter__()
            g_sb = pg.tile([P, DT, DX], f32r, tag="g", name="g_sb")

            # first Wq' quarter prefetched at body start so A-matmuls can
            # begin the moment G completes (the rest of Wq' loads into pc,
            # whose SBUF region only frees when xT dies)
            pwq0_cm = tc.tile_pool(name="pwq0", bufs=1)
            pwq0 = pwq0_cm.__enter__()
            wq_q0 = pwq0.tile([P, DT, 256], f32r, tag="wq0", name="wq_q0")

            pxt_cm = tc.tile_pool(name="pxt", bufs=1)
            pxt = pxt_cm.__enter__()
            xt = pxt.tile([P, NT, DX], f32r, tag="xt", name="xt")

            # gen0: G rows 0-1 x all cols, accumulated as chunks land
            GEN0 = [(0, 0), (0, 1), (1, 0), (1, 1)]   # (row, col-half)
            gp0 = [psg.tile([P, 512], f32, tag="psg", name=f"gp0_{m}")
                   for m in range(4)]

            ei = 0
            with (
                tc.tile_pool(name="pxa", bufs=3) as pxa,
                tc.tile_pool(name="pwq0st", bufs=1) as pwq0st,
            ):
                for c in range(NCA):
                    ncol = bass.ds(c * CHA, CHA)
                    xc = pxa.tile([P, DT, CHA], f32, tag="xc", name=f"xc{c}")
                    if c == 0:
                        nc.sync.dma_start(xc[:, :, 0:CHA // 2],
                                          xv[:, :, 0:CHA // 2])
                        nc.sync.dma_start(xc[:, :, CHA // 2:CHA],
                                          xv[:, :, CHA // 2:CHA])
                    else:
                        nc.sync.dma_start(xc[:], xv[:, :, ncol])
                    if c == 1:
                        w0tmp = pwq0st.tile([P, DT, 256], f32, tag="w0t",
                                            name="w0t")
                        nc.scalar.dma_start(w0tmp[:], wqv[:, :, 0:256])
                        nc.gpsimd.tensor_copy(wq_q0[:], w0tmp[:])
                    def gen0_mms(nt):
                        for m, (row, ch) in enumerate(GEN0):
                            nc.tensor.matmul(
                                gp0[m][:], xt[:, nt, row * P:(row + 1) * P],
                                xt[:, nt, ch * 512:(ch + 1) * 512],
                                start=(nt == 0), stop=(nt == NT - 1))

                    for j in range(CHA // P):
                        nt = c * (CHA // P) + j
                        for dh in range(DT // 4):
                            tp = ps.tile([P, 512], f32, tag="ps",
                                         name=f"tp{nt}_{dh}")
                            for di in range(4):
                                dt = dh * 4 + di
                                nc.tensor.transpose(
                                    tp[:, di * P:(di + 1) * P],
                                    xc[:, dt, j * P:(j + 1) * P],
                                    ident_f[:])
                            evict(ei, xt[:, nt, dh * 512:(dh + 1) * 512],
                                  tp[:])
                            ei += 1
                        if nt >= 1:
                            gen0_mms(nt - 1)
                        if nt == NT - 1:
                            gen0_mms(nt)

            # ------- Phase B: remaining G generations, mirror, g ----------
            for m, (row, ch) in enumerate(GEN0):
                evict(ei, g_sb[:, row, ch * 512:(ch + 1) * 512], gp0[m][:])
                ei += 1
            # upper-triangle ragged blocks (row, c0, c1); lower-left comes
            # from the mirror (G symmetric)
            GENS = [(2, 256, 768), (3, 256, 768), (4, 512, 1024),
                    (5, 512, 1024), (2, 768, 1024), (3, 768, 1024),
                    (6, 768, 1024), (7, 768, 1024)]
            for row, c0, c1 in GENS:
                gp = ps.tile([P, c1 - c0], f32, tag="ps",
                             name=f"gp{row}_{c0}")
                for nt in range(NT):
                    nc.tensor.matmul(
                        gp[:], xt[:, nt, row * P:(row + 1) * P],
                        xt[:, nt, bass.ds(c0, c1 - c0)],
                        start=(nt == 0), stop=(nt == NT - 1))
                evict(ei, g_sb[:, row, c0:c1], gp[:])
                ei += 1
            pxt_cm.__exit__(None, None, None)  # free xT (16MB)

            # mirror: G[r, c*128:] = G[c, r*128:]^T for tiles left of each
            # row's directly-computed range
            ROW_START = [0, 0, 2, 2, 4, 4, 6, 6]   # first computed col-tile
            for r in range(2, DT):
                cs = list(range(ROW_START[r]))
                for b0 in range(0, len(cs), 4):
                    grp = cs[b0:b0 + 4]
                    mp = ps.tile([P, len(grp) * P], f32r, tag="ps",
                                 name=f"mp{r}_{b0}")
                    for i, c in enumerate(grp):
                        nc.tensor.transpose(
                            mp[:, i * P:(i + 1) * P],
                            g_sb[:, c, r * P:(r + 1) * P],
                            ident_r[:])
                    evict(ei, g_sb[:, r, grp[0] * P:(grp[-1] + 1) * P],
                          mp[:])
                    ei += 1

            # g = rowsum(G) in two f32r limbs
            with tc.tile_pool(name="pgst", bufs=1) as pgst:
                g_f = pgst.tile([P, DT, 1], f32, tag="gf", name="g_f")
                for dt in range(DT):
                    nc.vector.reduce_sum(g_f[:, dt], g_sb[:, dt],
                                         axis=mybir.AxisListType.X)
                nc.vector.tensor_copy(g_hi[:], g_f[:])
                nc.vector.tensor_sub(g_lo[:], g_f[:], g_hi[:])

            # ------- Phase C: A, t, scores, softmax-from-PSUM -------------
            # ppr: tiles that outlive pc (p_r, wv_r used in E; wv prefetch
            # overlaps C because its buffer can't collide with pc tiles)
            ppr_cm = tc.tile_pool(name="ppr", bufs=1)
            ppr = ppr_cm.__enter__()
            p_r8 = ppr.tile([P, KT, DQ], f8, tag="pr8", name="p_r8")
            wv_r = ppr.tile([P, DT, DKH], f32r, tag="wv", name="wv_r")
            xe0 = ppr.tile([P, DT, ECH], f32, tag="xe0", name="xe0")
            nc.sync.dma_start(xe0[:], xv[:, :, 0:ECH])

            pc_cm = tc.tile_pool(name="pc", bufs=1)
            pc = pc_cm.__enter__()
            wq_r = pc.tile([P, DT, DQ - 256], f32r, tag="wq", name="wq_r")
            wk_r = pc.tile([P, DT, DKH], f32r, tag="wk", name="wk_r")
            a_sb = pc.tile([P, DT, DQ], f32r, tag="a", name="a_sb")
            t_hi = pc.tile([1, DQ], f32r, tag="thi", name="t_hi")
            t_lo = pc.tile([1, DQ], f32r, tag="tlo", name="t_lo")

            with tc.tile_pool(name="pwst", bufs=2) as pwst:
                def wq_ap(dt, q0, q1):
                    # Wq' col range [q0:q1): quarter 0 lives in wq_q0,
                    # the rest in wq_r at offset-256
                    if q1 <= 256:
                        return wq_q0[:, dt, q0:q1]
                    return wq_r[:, dt, q0 - 256:q1 - 256]

                for qq in range(1, DQ // 256):
                    wtmp = pwst.tile([P, DT, 256], f32, tag="wt",
                                     name=f"wt{qq}")
                    (nc.sync, nc.scalar, nc.gpsimd)[qq % 3].dma_start(
                        wtmp[:], wqv[:, :, qq * 256:(qq + 1) * 256])
                    nc.vector.tensor_copy(
                        wq_r[:, :, (qq - 1) * 256:qq * 256], wtmp[:])
                for dt in range(0, DT, 2):
                    d2 = bass.ds(dt, 2)
                    wtmp2 = pwst.tile([P, 2, DKH], f32, tag="wt2",
                                      name=f"wt2{dt}")
                    (nc.scalar, nc.gpsimd)[(dt // 2) % 2].dma_start(
                        wtmp2[:], wkv[:, d2])
                    nc.gpsimd.tensor_copy(wk_r[:, d2], wtmp2[:])
                    wtmp3 = pwst.tile([P, 2, DKH], f32, tag="wt3",
                                      name=f"wt3{dt}")
                    (nc.gpsimd, nc.sync)[(dt // 2) % 2].dma_start(
                        wtmp3[:], wvv[:, d2])
                    nc.gpsimd.tensor_copy(wv_r[:, d2], wtmp3[:])

                # A^T[d', q] = sum_d G[d, d'] Wq'^T[d, q], in 256-wide
                # quarters: quarter 0 uses the prefetched wq_q0 and starts
                # right at G-end, overlapping the wq_r load
                for qq in range(DQ // 256):
                    for dpt in reversed(range(DT)):
                        dsl = bass.ds(dpt * P, P)
                        ap_ = ps.tile([P, 256], f32, tag="ps",
                                      name=f"ap{dpt}_{qq}")
                        for dt in range(DT):
                            nc.tensor.matmul(
                                ap_[:], g_sb[:, dt, dsl],
                                wq_ap(dt, qq * 256, (qq + 1) * 256),
                                start=(dt == 0), stop=(dt == DT - 1))
                        evict(ei, a_sb[:, dpt, bass.ds(qq * 256, 256)],
                              ap_[:])
                        ei += 1

                # t[q] = 0.5 * (g_hi + g_lo)^T Wq'  (two f32r limbs)
                for qq in range(DQ // 256):
                    qsl = bass.ds(qq * 256, 256)
                    tp2 = ps.tile([P, 256], f32, tag="ps", name=f"tq{qq}")
                    first = True
                    for limb in (g_hi, g_lo):
                        for dt in range(DT):
                            nc.tensor.matmul(
                                tp2[0:1, :], limb[:, dt],
                                wq_ap(dt, qq * 256, (qq + 1) * 256),
                                start=first,
                                stop=(limb is g_lo and dt == DT - 1))
                            first = False
                    nc.scalar.mul(t_hi[:, qsl], tp2[0:1, :], 0.5)
                    nc.vector.scalar_tensor_tensor(
                        t_lo[:, qsl], tp2[0:1, :], 0.5, t_hi[:, qsl],
                        op0=mybir.AluOpType.mult,
                        op1=mybir.AluOpType.subtract)

            with (
                tc.tile_pool(name="psmx", bufs=2) as psmx,
                tc.tile_pool(name="pstat", bufs=4) as pstat,
            ):
                # scoresT[k, q] = Wk'^T A^T + ones x (t_hi + t_lo);
                # softmax over q straight from the two q-chunk PSUMs
                for kt in range(KT):
                    ksl = bass.ds(kt * P, P)
                    sp = []
                    for qc in range(QC):
                        qsl = bass.ds(qc * 512, 512)
                        s = psg.tile([P, 512], f32, tag="psg",
                                     name=f"sp{kt}_{qc}")
                        for dpt in range(DT):
                            nc.tensor.matmul(
                                s[:], wk_r[:, dpt, ksl], a_sb[:, dpt, qsl],
                                start=(dpt == 0), stop=False)
                        nc.tensor.matmul(s[:], ones_row[:], t_hi[:, qsl],
                                         start=False, stop=False)
                        nc.tensor.matmul(s[:], ones_row[:], t_lo[:, qsl],
                                         start=False, stop=True)
                        sp.append(s)
                    m0 = pstat.tile([P, 1], f32, tag="m0")
                    m1 = pstat.tile([P, 1], f32, tag="m1")
                    negm = pstat.tile([P, 1], f32, tag="negm")
                    den0 = pstat.tile([P, 1], f32, tag="den0")
                    den1 = pstat.tile([P, 1], f32, tag="den1")
                    rden = pstat.tile([P, 1], f32, tag="rden")
                    nc.vector.reduce_max(m0[:], sp[0][:],
                                         axis=mybir.AxisListType.X)
                    nc.vector.reduce_max(m1[:], sp[1][:],
                                         axis=mybir.AxisListType.X)
                    nc.vector.tensor_max(m0[:], m0[:], m1[:])
                    nc.vector.tensor_scalar_mul(negm[:], m0[:], -scale)
                    e0 = psmx.tile([P, 512], f32, tag="e0")
                    e1 = psmx.tile([P, 512], f32, tag="e1")
                    nc.scalar.activation(
                        e0[:], sp[0][:], mybir.ActivationFunctionType.Exp,
                        bias=negm[:], scale=scale, accum_out=den0[:])
                    nc.scalar.activation(
                        e1[:], sp[1][:], mybir.ActivationFunctionType.Exp,
                        bias=negm[:], scale=scale, accum_out=den1[:])
                    nc.vector.tensor_add(den0[:], den0[:], den1[:])
                    nc.vector.reciprocal(rden[:], den0[:])
                    nc.vector.tensor_scalar_mul(p_r8[:, kt, 0:512], e0[:],
                                                rden[:])
                    nc.vector.tensor_scalar_mul(p_r8[:, kt, 512:DQ], e1[:],
                                                rden[:])
            pc_cm.__exit__(None, None, None)

            # ------- Phase E: V proj fused with out -----------------------
            with (
                tc.tile_pool(name="pex", bufs=3) as pex,
                tc.tile_pool(name="pev", bufs=3) as pev,
                tc.tile_pool(name="pout", bufs=4) as pout,
                tc.tile_pool(name="pseed", bufs=1) as pseed,
            ):
                seed_sb = pseed.tile([1, 1], f32, tag="seed")
                nc.sync.dma_start(seed_sb[:], seed.ap())
                outv = out.ap().rearrange("(qt p) n -> p qt n", p=P)

                vs = {}

                def emit_v(c):
                    nonlocal ei
                    ncol2 = bass.ds(c * ECH, ECH)
                    if c == 0:
                        xc2 = xe0
                    else:
                        xc2 = pex.tile([P, DT, ECH], f32, tag="xc2",
                                       name=f"xe{c}")
                        nc.sync.dma_start(xc2[:], xv[:, :, ncol2])
                    xr2 = pex.tile([P, DT, ECH], f32r, tag="xr2",
                                   name=f"xre{c}")
                    nc.scalar.copy(xr2[:], xc2[:])
                    v_sb = pev.tile([P, KT, ECH], f8, tag="v", name=f"v{c}")
                    for vt in range(KT):
                        vp = ps.tile([P, ECH], f32, tag="ps",
                                     name=f"vp{c}_{vt}")
                        vsl = bass.ds(vt * P, P)
                        for dt in range(DT):
                            nc.tensor.matmul(
                                vp[:], wv_r[:, dt, vsl], xr2[:, dt],
                                start=(dt == 0), stop=(dt == DT - 1))
                        evict(ei, v_sb[:, vt], vp[:])
                        ei += 1
                    vs[c] = v_sb

                emit_v(0)
                for c in range(NCE):
                    ncol = bass.ds(c * ECH, ECH)
                    if c + 1 < NCE:
                        emit_v(c + 1)
                    v_sb = vs.pop(c)

                    for qg in range(QT128 // 4):
                        osb = pout.tile([P, 4, ECH], bf16, tag="osb")
                        for qi in range(4):
                            qt = qg * 4 + qi
                            op = psg.tile([P, ECH], f32, tag="psg",
                                          name=f"op{c}_{qt}")
                            qsl2 = bass.ds(qt * P, P)
                            for kp in range(KT // 2):
                                nc.tensor.matmul(
                                    op[:],
                                    p_r8[:, 2 * kp:2 * kp + 2, qsl2],
                                    v_sb[:, 2 * kp:2 * kp + 2, :],
                                    start=(kp == 0), stop=(kp == KT // 2 - 1),
                                    perf_mode=mybir.MatmulPerfMode.DoubleRow)
                            nc.vector.tensor_copy(osb[:, qi], op[:])
                            if c == 0 and qt == 0:
                                nc.vector.tensor_scalar_add(
                                    osb[0:1, 0, 0:1], op[0:1, 0:1],
                                    seed_sb[:])
                        nc.sync.dma_start(
                            outv[:, qg * 4:(qg + 1) * 4, ncol], osb[:])

            ppr_cm.__exit__(None, None, None)
            pwq0_cm.__exit__(None, None, None)
            pg_cm.__exit__(None, None, None)
            if rep_cm is not None:
                rep_cm.__exit__(None, None, None)
            if sink is not None:
                # touch every out-DMA region (walrus DCE is region-precise):
                # one full row per q-half covers all (qg, c) blocks
                with tc.tile_pool(name="psink", bufs=1) as psink:
                    row0 = psink.tile([1, N], bf16, tag="r0", name="row0")
                    row1 = psink.tile([1, N], bf16, tag="r1", name="row1")
                    nc.sync.dma_start(row0[:], out.ap()[0:1, :])
                    nc.sync.dma_start(row1[:], out.ap()[DQ // 2:DQ // 2 + 1, :])
                    s0 = psink.tile([1, 1], f32, tag="s0", name="s0")
                    s1 = psink.tile([1, 1], f32, tag="s1", name="s1")
                    nc.vector.reduce_sum(s0[:], row0[:],
                                         axis=mybir.AxisListType.X)
                    nc.vector.reduce_sum(s1[:], row1[:],
                                         axis=mybir.AxisListType.X)
                    nc.vector.tensor_add(s0[:], s0[:], s1[:])
                    nc.sync.dma_start(sink.ap(), s0[:])
            p0_cm.__exit__(None, None, None)

    nc.compile()
    return nc


_CACHE = {}


def _get_nc(DX, N, DQ, DKH):
    key = (DX, N, DQ, DKH)
    if key not in _CACHE:
        _CACHE[key] = _build_core_kernel(DX, N, DQ, DKH)
    return _CACHE[key]


def _run(x, Wq, Wk, Wv, **spmd_kwargs):
    from concourse.bass_utils import run_bass_kernel_spmd

    B, DX, N = x.shape
    DQ = Wq.shape[0]
    DK = Wk.shape[0]
    assert (B, DX, N, DQ, DK) == (B_FULL, DX_FULL, N_FULL, DQ_FULL, DK_FULL)
    DKH = DK // 2

    nc = _get_nc(DX, N, DQ, DKH)

    # Wq/Wk shipped mean-removed (entries - 0.5); the q-varying part of the
    # mean term is restored on-chip via t[q] (see module docstring)
    WqT = np.ascontiguousarray(Wq.T, dtype=np.float32) - np.float32(0.5)
    WkT = np.ascontiguousarray(Wk.T, dtype=np.float32) - np.float32(0.5)
    WvT = np.ascontiguousarray(Wv.T, dtype=np.float32)
    eye = np.eye(128, dtype=np.float32)

    in_maps = []
    for c in range(N_CORES):
        b, h = divmod(c, 2)
        hsl = slice(h * DKH, (h + 1) * DKH)
        in_maps.append({
            "xb": np.ascontiguousarray(x[b], dtype=np.float32),
            "wqt": WqT,
            "wkt": np.ascontiguousarray(WkT[:, hsl]),
            "wvt": np.ascontiguousarray(WvT[:, hsl]),
            "ident": eye,
            "seed": np.zeros((1, 1), np.float32),
        })

    res = run_bass_kernel_spmd(nc, in_maps, core_ids=list(range(N_CORES)),
                               **spmd_kwargs)
    out = np.empty((B, DQ, N), np.float32)
    for b in range(B):
        out[b] = (np.asarray(res.results[2 * b]["out"]).astype(np.float32)
                  + np.asarray(res.results[2 * b + 1]["out"]).astype(
                      np.float32))
    return out, res


def kernel(x, Wq, Wk, Wv):
    return _run(x, Wq, Wk, Wv)[0]
